# revision 1
# baseline (speedup 1.0000x reference)
# Trainium2 Bass kernel for DnCNN+S4D (nn_DnCNN_S4_74182675137230).
#
# Data parallel over batch B=64 across 8 NeuronCores (BL=8 per core).
# The S4D FFT long-conv is computed exactly via a chunked state-space scan
# (chunk C=128, stride-2 carry):
#   per channel h:   y_i       = T_h @ u_i                 (+ cross terms)
#                    sle[m]    = V @ u_{2m}
#                    pi[m]     = Z @ sle[m] + V @ u_{2m+1}
#                    c[m]      = z^{2C} (.) c[m-1] + pi[m]     (DVE scan)
#                    q[m]      = V @ u_{2m} + Z @ c[m-1]
#                    y_{2m+1} += M @ (sle[0] if m==0 else q[m])
#                    y_{2m+2} += M @ c[m]
# then gelu and the channel-mix Wout (+bout +residual); all matmuls fp16
# with fp32 PSUM accumulation.  T/V/M/Z are weight-only preprocessing
# (host-built from the S4 parameters, akin to BN folding / weight
# repacking); every batch-dependent FLOP runs on device.  Training-mode BN
# statistics are AllReduced across the 8 cores.
#
# Layouts (i2 = chunk parity i%2, ip = i//2, l = (2*ip+i2)*128 + c):
#   h-orient: [(i2,h)=128 part, (b=8, ip=8, c=128) free]   (convs, Wout, BN)
#   c-orient: [c=128 part, (b, ip, (i2,h)=128) free]       (per-h S4 matmuls)
# Orientation swaps are single xbar transpose DMAs:
#   dma_start_transpose: out[p, x, q] = in[q, 128*x + p].

import numpy as np

import concourse.bass as bass
import concourse.bacc as bacc
import concourse.tile as tile
from concourse import mybir
from concourse.bass_utils import run_bass_kernel_spmd

F32 = mybir.dt.float32
F16 = mybir.dt.float16
AF = mybir.ActivationFunctionType
OP = mybir.AluOpType

NCORES = 8
B, H, N, L, NB = 64, 64, 64, 2048, 13
BL = B // NCORES          # 8 local batches
C = 128                   # chunk length
NCH = L // C              # 16 chunks
IP = NCH // 2             # 8 chunk pairs (= carry steps)
KAP = 256.0               # state scaling to keep fp16 range
EPS = 1e-5
H2 = 2 * H                # 128 = (i2, h) partition extent
N2 = 2 * N                # 128 = (re/im, n) state extent


# ---------------------------------------------------------------------------
# Host-side weight preprocessing (numpy) -> fp16 device matrices
# ---------------------------------------------------------------------------

def _host_prep(inputs):
    out = {}
    log_dt = np.asarray(inputs['s4_log_dt'], np.float64)
    logA_re = np.asarray(inputs['s4_logA_re'], np.float64)
    A_im = np.asarray(inputs['s4_A_im'], np.float64)
    C_re = np.asarray(inputs['s4_C_re'], np.float64)
    C_im = np.asarray(inputs['s4_C_im'], np.float64)
    D = np.asarray(inputs['s4_D'], np.float64)
    Wout = np.asarray(inputs['s4_Wout'], np.float64)
    bout = np.asarray(inputs['s4_bout'], np.float64)

    dt = np.exp(log_dt)[:, :, None]
    A = -np.exp(logA_re) + 1j * A_im
    dtA = dt * A
    w = np.exp(dtA)                                            # (NB,H,N)
    Ct = (C_re + 1j * C_im) * (np.exp(dtA) - 1.0) / A

    cc = np.arange(C)
    P = w[..., None] ** np.arange(2 * C + 1)                   # (NB,H,N,2C+1)
    K = 2.0 * np.real(np.einsum('jhn,jhne->jhe', Ct, P[..., :C]))
    K[:, :, 0] += D                                            # D*u folded

    # T lhsT [c', (h, c)] with T[c,c'] = K[c-c']
    dmat = cc[None, :] - cc[:, None]                           # (c',c)
    Tl = np.where((dmat >= 0)[None, None],
                  np.take_along_axis(np.broadcast_to(K[:, :, None, :],
                                                     (NB, H, C, C)),
                                     np.clip(dmat, 0, C - 1)[None, None],
                                     axis=3), 0.0)             # (NB,H,c',c)
    out['tmat'] = np.ascontiguousarray(
        Tl.transpose(2, 0, 1, 3).reshape(C, NB, H * C), np.float16)

    # V lhsT [c', (h, 2n)]: V[(ri,n),c'] = [Re;Im](Ct w^(C-1-c'))/KAP
    VC = Ct[..., None] * P[..., (C - 1) - cc]                  # (NB,H,N,c')
    Vl = np.concatenate([VC.real, VC.imag], axis=2) / KAP      # (NB,H,2N,c')
    out['vmat'] = np.ascontiguousarray(
        Vl.transpose(3, 0, 1, 2).reshape(C, NB, H * N2), np.float16)

    # M lhsT [(ri,n), (h, c)]: y_cross[c] = 2*KAP*[Re|-Im](w^{c+1}) . s
    MC = P[..., cc + 1]                                        # (NB,H,N,c)
    Ml = np.concatenate([2 * KAP * MC.real, -2 * KAP * MC.imag], axis=2)
    out['mmat'] = np.ascontiguousarray(
        Ml.transpose(0, 2, 1, 3).reshape(NB, N2, H * C).transpose(1, 0, 2),
        np.float16)

    # Z lhsT [(ri',n'), (h, (ro,n))]: out_re = zr*s_re - zi*s_im, z = w^C
    zC = w ** C
    Zl = np.zeros((NB, H, N2, N2))
    nn = np.arange(N)
    Zl[:, :, nn, nn] = zC.real;       Zl[:, :, N + nn, nn] = -zC.imag
    Zl[:, :, nn, N + nn] = zC.imag;   Zl[:, :, N + nn, N + nn] = zC.real
    out['zmat'] = np.ascontiguousarray(
        Zl.transpose(2, 0, 1, 3).reshape(N2, NB, H * N2), np.float16)

    # scan multipliers z2 = w^{2C}: tiles [(ri,n), NB, h]
    z2 = w ** (2 * C)
    zr = z2.real.transpose(0, 2, 1)                            # (NB,N,H)
    zi = z2.imag.transpose(0, 2, 1)
    out['zrscan'] = np.ascontiguousarray(
        np.concatenate([zr, zr], 1).transpose(1, 0, 2), np.float16)
    out['ziscan'] = np.ascontiguousarray(
        np.concatenate([zi, -zi], 1).transpose(1, 0, 2), np.float16)

    # Wout block-diag over i2: lhsT[(i2,h),(i2',o)] = Wout[o,h] d_{i2,i2'}
    wblk = np.zeros((NB, H2, H2))
    WT = Wout.transpose(0, 2, 1)
    wblk[:, :H, :H] = WT; wblk[:, H:, H:] = WT
    out['wblk'] = np.ascontiguousarray(wblk.transpose(1, 0, 2), np.float16)
    out['bout2'] = np.ascontiguousarray(
        np.concatenate([bout, bout], 1).T, np.float32)         # [128, NB]

    # conv stationaries: per tap k block-diag over i2, plus i2-crossing
    # edge stationaries (out c=0 needs l-1 from the other parity, etc).
    def conv_stat(Wc):                                         # (O, Hin, 3)
        O = Wc.shape[0]; Hin = Wc.shape[1]
        res = {}
        for k in range(3):
            Wk = np.zeros((2 * Hin, 2 * O))
            Wk[:Hin, :O] = Wc[:, :, k].T; Wk[Hin:, O:] = Wc[:, :, k].T
            res[f'k{k}'] = Wk
        E0a = np.zeros((2 * Hin, 2 * O)); E0a[Hin:, :O] = Wc[:, :, 0].T
        E0b = np.zeros((2 * Hin, 2 * O)); E0b[:Hin, O:] = Wc[:, :, 0].T
        E2a = np.zeros((2 * Hin, 2 * O)); E2a[Hin:, :O] = Wc[:, :, 2].T
        E2b = np.zeros((2 * Hin, 2 * O)); E2b[:Hin, O:] = Wc[:, :, 2].T
        res.update(e0a=E0a, e0b=E0b, e2a=E2a, e2b=E2b)
        return {k_: v.astype(np.float16) for k_, v in res.items()}
    convall = np.zeros((128, 28, 128), np.float16)
    for ni, (nm, key) in enumerate((('c1', 'conv1_w'), ('c9', 'conv9_w'),
                                    ('c16', 'conv16_w'), ('c17', 'conv17_w'))):
        cs = conv_stat(np.asarray(inputs[key], np.float64))
        for si, sfx in enumerate(('k0', 'k1', 'k2', 'e0a', 'e0b', 'e2a', 'e2b')):
            arr = cs[sfx]
            convall[:arr.shape[0], ni * 7 + si, :arr.shape[1]] = arr
    out['convall'] = np.ascontiguousarray(convall.reshape(128, 28 * 128))

    bn = np.zeros((H, 6), np.float32)
    for k, nm in enumerate(('bn1', 'bn9', 'bn16')):
        bn[:, 2 * k] = np.asarray(inputs[nm + '_g'], np.float32)
        bn[:, 2 * k + 1] = np.asarray(inputs[nm + '_b'], np.float32)
    out['bnp'] = bn
    return out


def _host_prep_x(x_shard):
    # x (BL,1,L) -> [(i2, ci=1)=2 part, (b, ip, c)] fp16
    xs = np.asarray(x_shard, np.float32).reshape(BL, IP, 2, C)
    return np.ascontiguousarray(xs.transpose(2, 0, 1, 3), np.float16)


# ---------------------------------------------------------------------------
# Device kernel
# ---------------------------------------------------------------------------

_CACHE = {}

CONV_SFX = ('k0', 'k1', 'k2', 'e0a', 'e0b', 'e2a', 'e2b')


def _conv_layer(nc, P_ps, src, dst_raw, stats_out, w, dma_out=None):
    """conv k=3 pad=1 from h-orient src into raw (pre-BN) dst + bn_stats.

    src: [(i2,h) or 2, (b, ip, c)];  dst_raw: [(i2,o)-rows, (b, ip, c)].
    16 psum chunks of (b, half-of-ip).
    """
    nparts = 2 if dma_out is not None else dst_raw.shape[0]
    for b in range(BL):
        for iph in range(2):
            ip0 = 4 * iph
            ips = slice(ip0, ip0 + 4)
            acc = P_ps.tile([nparts, 4, C], F32, tag="bigps")
            nc.tensor.matmul(acc[:, :, :], w['k1'], src[:, b, ips, :],
                             start=True, stop=False)
            nc.tensor.matmul(acc[:, :, 1:C], w['k0'],
                             src[:, b, ips, 0:C - 1], start=False, stop=False)
            nc.tensor.matmul(acc[:, :, 0:C - 1], w['k2'],
                             src[:, b, ips, 1:C], start=False, stop=False)
            # k0-edge: out c=0 <- src (other parity) c=127
            if ip0 == 0:
                nc.tensor.matmul(acc[:, 1:4, 0:1], w['e0a'],
                                 src[:, b, 0:3, C - 1:C],
                                 start=False, stop=False)
            else:
                nc.tensor.matmul(acc[:, 0:4, 0:1], w['e0a'],
                                 src[:, b, ip0 - 1:ip0 + 3, C - 1:C],
                                 start=False, stop=False)
            nc.tensor.matmul(acc[:, 0:4, 0:1], w['e0b'],
                             src[:, b, ips, C - 1:C], start=False, stop=False)
            # k2-edge: out c=127 <- src (other parity) c=0
            nc.tensor.matmul(acc[:, 0:4, C - 1:C], w['e2a'],
                             src[:, b, ips, 0:1], start=False, stop=False)
            if ip0 == 0:
                nc.tensor.matmul(acc[:, 0:4, C - 1:C], w['e2b'],
                                 src[:, b, 1:5, 0:1], start=False, stop=True)
            else:
                nc.tensor.matmul(acc[:, 0:3, C - 1:C], w['e2b'],
                                 src[:, b, ip0 + 1:ip0 + 4, 0:1],
                                 start=False, stop=True)
            if stats_out is not None:
                nc.vector.bn_stats(out=stats_out[:, 2 * b + iph, :],
                                   in_=acc.rearrange("p a c -> p (a c)"))
            if dma_out is not None:
                ev = dma_out[1].tile([2, 4, C], F32, tag="finev")
                nc.scalar.activation(out=ev, in_=acc, func=AF.Copy)
                nc.sync.dma_start(out=dma_out[0][:, b, ips, :], in_=ev)
            else:
                nc.scalar.activation(out=dst_raw[:, b, ips, :], in_=acc,
                                     func=AF.Copy)


def _bn_finalize(nc, scr, P_dram, stats, bnp, k, gb128):
    """stats [128,16,6] --bn_aggr/AllReduce--> gb128 [128,2]=(gamma',beta').

    scr: packed scratch tile [H2, 32] F32."""
    mv = scr[:, 0:2]; pay = scr[:, 2:4]; red = scr[:, 4:6]
    pair = scr[0:H, 6:10].rearrange("p (x y) -> p x y", x=2)
    gm = scr[0:H, 10:12]; var = scr[0:H, 12:13]; eps_t = scr[0:H, 13:14]
    std = scr[0:H, 14:15]; rs = scr[0:H, 15:16]; gp = scr[0:H, 16:18]
    nc.vector.bn_aggr(out=mv, in_=stats)
    nc.vector.tensor_tensor(out=pay[:, 1:2], in0=mv[:, 0:1], in1=mv[:, 0:1],
                            op=OP.mult)
    nc.vector.tensor_tensor(out=pay[:, 1:2], in0=pay[:, 1:2], in1=mv[:, 1:2],
                            op=OP.add)
    nc.vector.tensor_copy(out=pay[:, 0:1], in_=mv[:, 0:1])
    cin = P_dram.tile([H2, 2], F32, tag=f"bnin{k}")
    cout = P_dram.tile([H2, 2], F32, tag=f"bnout{k}", addr_space="Shared")
    nc.sync.dma_start(out=cin, in_=pay)
    nc.gpsimd.collective_compute("AllReduce", OP.add, ins=[cin[:]],
                                 outs=[cout[:]],
                                 replica_groups=[list(range(NCORES))])
    nc.sync.dma_start(out=red, in_=cout)
    nc.sync.dma_start(out=pair[:, 0, :], in_=red[0:H, :])
    nc.sync.dma_start(out=pair[:, 1, :], in_=red[H:H2, :])
    nc.vector.tensor_tensor(out=gm, in0=pair[:, 0, :], in1=pair[:, 1, :],
                            op=OP.add)
    nc.vector.tensor_scalar_mul(gm, gm, 1.0 / (2 * NCORES))
    nc.vector.tensor_tensor(out=var, in0=gm[:, 0:1], in1=gm[:, 0:1],
                            op=OP.mult)
    nc.vector.tensor_tensor(out=var, in0=gm[:, 1:2], in1=var, op=OP.subtract)
    nc.vector.memset(eps_t, EPS)
    nc.scalar.activation(out=std, in_=var, func=AF.Sqrt, bias=eps_t)
    nc.vector.reciprocal(out=rs, in_=std)
    nc.vector.tensor_tensor(out=gp[:, 0:1], in0=bnp[:, 2 * k:2 * k + 1],
                            in1=rs, op=OP.mult)
    nc.vector.tensor_tensor(out=gp[:, 1:2], in0=gp[:, 0:1], in1=gm[:, 0:1],
                            op=OP.mult)
    nc.vector.tensor_tensor(out=gp[:, 1:2], in0=bnp[:, 2 * k + 1:2 * k + 2],
                            in1=gp[:, 1:2], op=OP.subtract)
    nc.sync.dma_start(out=gb128[0:H, :], in_=gp)
    nc.sync.dma_start(out=gb128[H:H2, :], in_=gp)


def _build():
    if 'nc' in _CACHE:
        return _CACHE['nc']
    import contextlib
    nc = bacc.Bacc("TRN2", target_bir_lowering=False, debug=False,
                   num_devices=NCORES)

    dram = {}
    def din(name, shape, dtype=F16):
        dram[name] = nc.dram_tensor(name, shape, dtype,
                                    kind="ExternalInput").ap()

    din('xq', [2, BL, IP, C])
    din('tmat', [C, NB, H * C]); din('vmat', [C, NB, H * N2])
    din('zmat', [N2, NB, H * N2]); din('mmat', [N2, NB, H * C])
    din('zrscan', [N2, NB, H]); din('ziscan', [N2, NB, H])
    din('wblk', [H2, NB, H2]); din('bout2', [H2, NB], F32)
    din('convall', [128, 28 * 128])
    din('bnp', [H, 6], F32)
    out_d = nc.dram_tensor('out', [2, BL, IP, C], F32,
                           kind="ExternalOutput").ap()

    with tile.TileContext(nc) as tc:
        ctx = contextlib.ExitStack()
        P_stat = ctx.enter_context(tc.tile_pool(name="stat", bufs=1))
        P_act = ctx.enter_context(tc.tile_pool(name="act", bufs=1))
        P_w = ctx.enter_context(tc.tile_pool(name="wstream", bufs=2))
        P_wz = ctx.enter_context(tc.tile_pool(name="wz", bufs=1))
        P_tmp = ctx.enter_context(tc.tile_pool(name="tmp", bufs=2))
        P_ps = ctx.enter_context(tc.tile_pool(name="ps", bufs=2, space="PSUM"))
        P_ps_y = ctx.enter_context(tc.tile_pool(name="psy", bufs=2,
                                                space="PSUM"))
        P_ps_s = ctx.enter_context(tc.tile_pool(name="pss", bufs=2,
                                                space="PSUM"))
        P_dram = ctx.enter_context(tc.tile_pool(name="cdram", bufs=1,
                                                space="DRAM"))

        # static tiles
        zr_s = P_stat.tile([N2, NB, H], F16)
        nc.sync.dma_start(out=zr_s, in_=dram['zrscan'])
        zi_s = P_stat.tile([N2, NB, H], F16)
        nc.sync.dma_start(out=zi_s, in_=dram['ziscan'])
        wblk_s = P_stat.tile([H2, NB, H2], F16)
        nc.sync.dma_start(out=wblk_s, in_=dram['wblk'])
        bout_s = P_stat.tile([H2, NB], F32)
        nc.sync.dma_start(out=bout_s, in_=dram['bout2'])
        bnp_s = P_stat.tile([H, 6], F32)
        nc.sync.dma_start(out=bnp_s, in_=dram['bnp'])
        xq_s = P_stat.tile([2, BL, IP, C], F16)
        nc.sync.dma_start(out=xq_s, in_=dram['xq'])
        convall_s = P_stat.tile([128, 28, 128], F16)
        nc.sync.dma_start(out=convall_s, in_=dram['convall'].rearrange(
            "p (k c) -> p k c", k=28))
        convw = {}
        for ni, nm in enumerate(('c1', 'c9', 'c16', 'c17')):
            npart = 2 if nm == 'c1' else H2
            ncol = 2 if nm == 'c17' else H2
            convw[nm] = {sfx: convall_s[0:npart, ni * 7 + si, 0:ncol]
                         for si, sfx in enumerate(CONV_SFX)}

        # activations
        uT = P_act.tile([H2, BL, IP, C], F16, tag="uT")
        uTn = P_act.tile([H2, BL, IP, C], F16, tag="uTn")
        u16c = P_act.tile([C, BL, IP, H2], F16, tag="u16c")
        g16c = P_act.tile([C, BL, IP, H2], F16, tag="g16c")
        gT = u16c  # same bytes; [H2,(b,ip,c)] view vs [c,(b,ip,(i2,h))]
        sle_t = P_act.tile([N2, IP, H, BL], F16, tag="sle")
        pi_t = P_act.tile([N2, IP, H, BL], F16, tag="pi")
        cst_t = P_act.tile([N2, IP, H, BL], F16, tag="cst")
        q_t = P_act.tile([N2, IP, H, BL], F16, tag="qt")
        gb128 = P_stat.tile([H2, 2], F32, tag="gb128")

        HG = 16                                   # h-group for tm/mm stream

        def s4_block(j, u_h, u_next):
            # transpose h-orient input into c-orient
            nc.sync.dma_start_transpose(
                u16c.rearrange("c b i p -> c (b i) p"),
                u_h.rearrange("p b i c -> p (b i c)"))
            vm = P_wz.tile([C, H, N2], F16, tag="vm")
            nc.sync.dma_start(out=vm, in_=dram['vmat'][:, j].rearrange(
                "p (h n) -> p h n", h=H))
            zm = P_wz.tile([N2, H, N2], F16, tag="zm")
            nc.sync.dma_start(out=zm, in_=dram['zmat'][:, j].rearrange(
                "p (h n) -> p h n", h=H))
            u5 = u16c.rearrange("c b i (x h) -> c b i x h", x=2)

            # ---- sle (even chunks) and pi, 8-h-batched psums ----
            for hg in range(H // 8):
                acc = P_ps_s.tile([N2, IP, 8, BL], F32, tag="spsum")
                for hh in range(8):
                    h = 8 * hg + hh
                    mv = u5[:, :, :, 0, h].rearrange("c b i -> c i b")
                    nc.tensor.matmul(acc[:, :, hh, :], vm[:, h, :], mv,
                                     start=True, stop=True)
                nc.scalar.activation(out=sle_t[:, :, 8 * hg:8 * hg + 8, :],
                                     in_=acc, func=AF.Copy)
            for hg in range(H // 8):
                acc = P_ps_s.tile([N2, IP, 8, BL], F32, tag="spsum")
                for hh in range(8):
                    h = 8 * hg + hh
                    nc.tensor.matmul(acc[:, :, hh, :], zm[:, h, :],
                                     sle_t[:, :, h, :], start=True, stop=False)
                    mv = u5[:, :, :, 1, h].rearrange("c b i -> c i b")
                    nc.tensor.matmul(acc[:, :, hh, :], vm[:, h, :], mv,
                                     start=False, stop=True)
                nc.scalar.activation(out=pi_t[:, :, 8 * hg:8 * hg + 8, :],
                                     in_=acc, func=AF.Copy)
            # ---- carry scan ----
            nc.vector.tensor_copy(out=cst_t[:, 0], in_=pi_t[:, 0])
            zr_b = zr_s[:, j, :]
            zr_ap = bass.AP(tensor=zr_b.tensor, offset=zr_b.offset,
                            ap=[zr_b.ap[0], zr_b.ap[1], [0, BL]])
            zi_b = zi_s[0:N, j, :]          # +zi at base 0
            zi_lo = bass.AP(tensor=zi_b.tensor, offset=zi_b.offset,
                            ap=[zi_b.ap[0], zi_b.ap[1], [0, BL]])
            zi_c = zi_s[N:N2, j, :]         # -zi at base 64
            zi_hi = bass.AP(tensor=zi_c.tensor, offset=zi_c.offset,
                            ap=[zi_c.ap[0], zi_c.ap[1], [0, BL]])
            for m in range(1, IP):
                tsw = P_tmp.tile([N2, H, BL], F16, tag="tsw")
                tzr = P_tmp.tile([N2, H, BL], F16, tag="tzr")
                nc.vector.tensor_tensor(out=tsw[0:N], in0=cst_t[N:, m - 1],
                                        in1=zi_hi, op=OP.mult)
                nc.vector.tensor_tensor(out=tsw[N:], in0=cst_t[0:N, m - 1],
                                        in1=zi_lo, op=OP.mult)
                nc.vector.tensor_tensor(out=tzr, in0=cst_t[:, m - 1],
                                        in1=zr_ap, op=OP.mult)
                nc.vector.tensor_tensor(out=tzr, in0=tzr, in1=tsw, op=OP.add)
                nc.vector.tensor_tensor(out=cst_t[:, m], in0=tzr,
                                        in1=pi_t[:, m], op=OP.add)
            # ---- q[m] = V@u_even[m] + Z@c[m-1], m=1..7 ----
            for hg in range(H // 8):
                acc = P_ps_s.tile([N2, IP, 8, BL], F32, tag="spsum")
                for hh in range(8):
                    h = 8 * hg + hh
                    mv = u5[:, :, 1:IP, 0, h].rearrange("c b i -> c i b")
                    nc.tensor.matmul(acc[:, 1:IP, hh, :], vm[:, h, :], mv,
                                     start=True, stop=False)
                    nc.tensor.matmul(acc[:, 1:IP, hh, :], zm[:, h, :],
                                     cst_t[:, 0:IP - 1, h, :],
                                     start=False, stop=True)
                nc.scalar.activation(out=q_t[:, 1:IP, 8 * hg:8 * hg + 8, :],
                                     in_=acc[:, 1:IP], func=AF.Copy)
            # ---- y psum (T@u + cross), 4-h batched; gelu evict ----
            for hgi in range(H // HG):
                tm = P_w.tile([C, HG, C], F16, tag="tm")
                nc.sync.dma_start(out=tm, in_=dram['tmat'][:, j].rearrange(
                    "p (h c) -> p h c", h=H)[:, HG * hgi:HG * (hgi + 1), :])
                mm = P_w.tile([N2, HG, C], F16, tag="mm")
                nc.sync.dma_start(out=mm, in_=dram['mmat'][:, j].rearrange(
                    "p (h c) -> p h c", h=H)[:, HG * hgi:HG * (hgi + 1), :])
                for hq in range(HG // 4):
                    acc = P_ps_y.tile([C, BL, IP, 2, 4], F32, tag="yps")
                    for hh in range(4):
                        hl = 4 * hq + hh
                        h = HG * hgi + hl
                        nc.tensor.matmul(acc[:, :, :, :, hh], tm[:, hl, :],
                                         u5[:, :, :, :, h],
                                         start=True, stop=False)
                        nc.tensor.matmul(
                            acc[:, :, 0, 1, hh], mm[:, hl, :],
                            sle_t[:, 0, h, :], start=False, stop=False)
                        nc.tensor.matmul(
                            acc[:, :, 1:IP, 1, hh], mm[:, hl, :],
                            q_t[:, 1:IP, h, :].rearrange("p m b -> p b m"),
                            start=False, stop=False)
                        nc.tensor.matmul(
                            acc[:, :, 1:IP, 0, hh], mm[:, hl, :],
                            cst_t[:, 0:IP - 1, h, :].rearrange(
                                "p m b -> p b m"),
                            start=False, stop=True)
                    dst = g16c.rearrange("c b i (x h) -> c (b i) x h", x=2)[
                        :, :, :, HG * hgi + 4 * hq:HG * hgi + 4 * hq + 4]
                    nc.scalar.activation(
                        out=dst,
                        in_=acc.rearrange("c b i x h -> c (b i) x h"),
                        func=AF.Gelu)
            # ---- gT via xbar; Wout + bout + residual ----
            nc.sync.dma_start_transpose(
                gT.rearrange("p b i c -> p (b i) c"),
                g16c.rearrange("c b i p -> c (b i p)"))
            gT_f = gT.rearrange("p b i c -> p (b i c)")
            uh_f = u_h.rearrange("p b i c -> p (b i c)")
            un_f = u_next.rearrange("p b i c -> p (b i c)")
            for t in range(16):
                sl = slice(512 * t, 512 * (t + 1))
                acc = P_ps.tile([H2, 512], F32, tag="bigps")
                nc.tensor.matmul(acc, wblk_s[:, j, :], gT_f[:, sl],
                                 start=True, stop=True)
                nc.vector.scalar_tensor_tensor(
                    out=un_f[:, sl], in0=acc, scalar=bout_s[:, j:j + 1],
                    in1=uh_f[:, sl], op0=OP.add, op1=OP.add)

        # ---------------- network ----------------
        bnscr = P_stat.tile([H2, 32], F32, tag="bnscr")
        stats = P_stat.tile([H2, 16, 6], F32, tag="stats")
        _conv_layer(nc, P_ps, xq_s, gT, stats, convw['c1'])
        _bn_finalize(nc, bnscr, P_dram, stats, bnp_s, 0, gb128)
        nc.scalar.activation(out=uT, in_=gT, func=AF.Relu,
                             bias=gb128[:, 1:2], scale=gb128[:, 0:1])
        cur, nxt = uT, uTn
        for j in range(7):
            s4_block(j, cur, nxt)
            cur, nxt = nxt, cur
        _conv_layer(nc, P_ps, cur, gT, stats, convw['c9'])
        _bn_finalize(nc, bnscr, P_dram, stats, bnp_s, 1, gb128)
        nc.scalar.activation(out=cur, in_=gT, func=AF.Relu,
                             bias=gb128[:, 1:2], scale=gb128[:, 0:1])
        for j in range(7, 13):
            s4_block(j, cur, nxt)
            cur, nxt = nxt, cur
        _conv_layer(nc, P_ps, cur, gT, stats, convw['c16'])
        _bn_finalize(nc, bnscr, P_dram, stats, bnp_s, 2, gb128)
        nc.scalar.activation(out=cur, in_=gT, func=AF.Relu,
                             bias=gb128[:, 1:2], scale=gb128[:, 0:1])
        _conv_layer(nc, P_ps, cur, None, None, convw['c17'], dma_out=(out_d, P_tmp))
        ctx.close()

    nc.compile()
    _CACHE['nc'] = nc
    return nc


# ---------------------------------------------------------------------------
# Entry point
# ---------------------------------------------------------------------------

def kernel(**inputs):
    nc = _build()
    prep = _host_prep(inputs)
    x = np.asarray(inputs['x'], np.float32)
    in_maps = []
    for c in range(NCORES):
        m = dict(prep)
        m['xq'] = _host_prep_x(x[c * BL:(c + 1) * BL])
        in_maps.append(m)
    res = run_bass_kernel_spmd(nc, in_maps, core_ids=list(range(NCORES)))
    outs = []
    for c in range(NCORES):
        o = res.results[c]['out']              # [2, BL, IP, C]
        outs.append(o.transpose(1, 2, 0, 3).reshape(BL, 1, L))
    return np.ascontiguousarray(np.concatenate(outs, 0), np.float32)



# revision 3
# speedup vs baseline: 1.0743x; 1.0743x over previous
# Trainium2 Bass kernel for DnCNN+S4D (nn_DnCNN_S4_74182675137230).
#
# Data parallel over batch B=64 across 8 NeuronCores (BL=8 per core).
# The S4D FFT long-conv is computed exactly via a chunked state-space scan
# (chunk C=128, stride-2 carry):
#   per channel h:   sle[m]  = V @ u_{2m}                     (even states)
#                    pi[m]   = (Z V) @ u_{2m} ... wait, see below
#                    pi[m]   = (ZV) @ u_{2m} + V @ u_{2m+1}   (pair partial)
#                    c[m]    = z^{2C} (.) c[m-1] + pi[m]      (DVE scan)
#                    y_l     = T @ u_l                        (local Toeplitz)
#                    y_{2m+1}+= M @ sle[m] + M2 @ c[m-1]      (M2 = M z^C)
#                    y_{2m}  += M @ c[m-1]
# then gelu and the channel-mix Wout (+bout +residual); all matmuls fp16
# with fp32 PSUM accumulation.  T/V/ZV/M/M2 are weight-only host
# preprocessing.  Training-mode BN statistics are AllReduced across cores.
#
# State slots are interleaved [sle0, c0, sle1, c1, ..., sle7, pad] so the
# full cross contribution streams as a single 120-column matmul per h.
#
# Layouts (x = chunk parity i%2, ip = i//2, l = 2*ip+x, pos = l*128 + c):
#   h-orient: [(x,h)=128 part, (b=8, ip=8, c=128) free]   (convs, Wout, BN)
#   c-orient: [c=128 part, (b, ip, (x,h)=128) free]       (per-h S4 matmuls)
# Orientation swaps are xbar transpose DMAs chunked by b-pairs.

import numpy as np

import concourse.bass as bass
import concourse.bacc as bacc
import concourse.tile as tile
from concourse import mybir
from concourse.bass_utils import run_bass_kernel_spmd

F32 = mybir.dt.float32
F16 = mybir.dt.float16
AF = mybir.ActivationFunctionType
OP = mybir.AluOpType

NCORES = 8
B, H, N, L, NB = 64, 64, 64, 2048, 13
BL = B // NCORES          # 8 local batches
C = 128                   # chunk length
NCH = L // C              # 16 chunks
IP = NCH // 2             # 8 chunk pairs (= carry steps)
KAP = 256.0               # state scaling to keep fp16 range
EPS = 1e-5
H2 = 2 * H                # 128 = (x, h) partition extent
N2 = 2 * N                # 128 = (re/im, n) state extent
HC = 16                   # h-chunk for weight streaming


# ---------------------------------------------------------------------------
# Host-side weight preprocessing (numpy) -> fp16 device matrices
# ---------------------------------------------------------------------------

def _host_prep(inputs):
    out = {}
    log_dt = np.asarray(inputs['s4_log_dt'], np.float64)
    logA_re = np.asarray(inputs['s4_logA_re'], np.float64)
    A_im = np.asarray(inputs['s4_A_im'], np.float64)
    C_re = np.asarray(inputs['s4_C_re'], np.float64)
    C_im = np.asarray(inputs['s4_C_im'], np.float64)
    D = np.asarray(inputs['s4_D'], np.float64)
    Wout = np.asarray(inputs['s4_Wout'], np.float64)
    bout = np.asarray(inputs['s4_bout'], np.float64)

    dt = np.exp(log_dt)[:, :, None]
    A = -np.exp(logA_re) + 1j * A_im
    dtA = dt * A
    w = np.exp(dtA)                                            # (NB,H,N)
    Ct = (C_re + 1j * C_im) * (np.exp(dtA) - 1.0) / A

    cc = np.arange(C)
    P = w[..., None] ** np.arange(2 * C + 1)                   # (NB,H,N,2C+1)
    K = 2.0 * np.real(np.einsum('jhn,jhne->jhe', Ct, P[..., :C]))
    K[:, :, 0] += D                                            # D*u folded

    # T lhsT [c', (h, c)] with T[c,c'] = K[c-c']
    dmat = cc[None, :] - cc[:, None]                           # (c',c)
    Tl = np.where((dmat >= 0)[None, None],
                  np.take_along_axis(np.broadcast_to(K[:, :, None, :],
                                                     (NB, H, C, C)),
                                     np.clip(dmat, 0, C - 1)[None, None],
                                     axis=3), 0.0)             # (NB,H,c',c)
    out['tmat'] = np.ascontiguousarray(
        Tl.transpose(2, 0, 1, 3).reshape(C, NB, H * C), np.float16)

    # V lhsT [c', (h, 2n)]: V[(ri,n),c'] = [Re;Im](Ct w^(C-1-c'))/KAP
    VC = Ct[..., None] * P[..., (C - 1) - cc]                  # (NB,H,N,c')
    Vl = np.concatenate([VC.real, VC.imag], axis=2) / KAP      # (NB,H,2N,c')
    out['vmat'] = np.ascontiguousarray(
        Vl.transpose(3, 0, 1, 2).reshape(C, NB, H * N2), np.float16)

    # ZV lhsT: (z^C V) -> Ct w^(2C-1-c')/KAP
    ZC = Ct[..., None] * P[..., (2 * C - 1) - cc]              # (NB,H,N,c')
    Zl = np.concatenate([ZC.real, ZC.imag], axis=2) / KAP
    out['zvmat'] = np.ascontiguousarray(
        Zl.transpose(3, 0, 1, 2).reshape(C, NB, H * N2), np.float16)

    # M lhsT [(ri,n), (h, c)]: y_cross[c] = 2*KAP*[Re|-Im](w^{c+1}) . s
    MC = P[..., cc + 1]                                        # (NB,H,N,c)
    Ml = np.concatenate([2 * KAP * MC.real, -2 * KAP * MC.imag], axis=2)
    out['mmat'] = np.ascontiguousarray(
        Ml.transpose(0, 2, 1, 3).reshape(NB, N2, H * C).transpose(1, 0, 2),
        np.float16)

    # M2 = M z^C: w^{c+1+C}
    M2C = P[..., cc + 1 + C]                                   # (NB,H,N,c)
    M2l = np.concatenate([2 * KAP * M2C.real, -2 * KAP * M2C.imag], axis=2)
    out['m2mat'] = np.ascontiguousarray(
        M2l.transpose(0, 2, 1, 3).reshape(NB, N2, H * C).transpose(1, 0, 2),
        np.float16)

    # scan multipliers z2 = w^{2C}: tiles [(ri,n), NB, h]
    z2 = w ** (2 * C)
    zr = z2.real.transpose(0, 2, 1)                            # (NB,N,H)
    zi = z2.imag.transpose(0, 2, 1)
    out['zrscan'] = np.ascontiguousarray(
        np.concatenate([zr, zr], 1).transpose(1, 0, 2), np.float16)
    out['ziscan'] = np.ascontiguousarray(
        np.concatenate([zi, -zi], 1).transpose(1, 0, 2), np.float16)

    # Wout block-diag over x: lhsT[(x,h),(x',o)] = Wout[o,h] d_{x,x'}
    wblk = np.zeros((NB, H2, H2))
    WT = Wout.transpose(0, 2, 1)
    wblk[:, :H, :H] = WT; wblk[:, H:, H:] = WT
    out['wblk'] = np.ascontiguousarray(wblk.transpose(1, 0, 2), np.float16)
    out['bout2'] = np.ascontiguousarray(
        np.concatenate([bout, bout], 1).T, np.float32)         # [128, NB]

    # conv stationaries: per tap k block-diag over x, plus x-crossing
    # edge stationaries (out c=0 needs l-1 from the other parity, etc).
    def conv_stat(Wc):                                         # (O, Hin, 3)
        O = Wc.shape[0]; Hin = Wc.shape[1]
        res = {}
        for k in range(3):
            Wk = np.zeros((2 * Hin, 2 * O))
            Wk[:Hin, :O] = Wc[:, :, k].T; Wk[Hin:, O:] = Wc[:, :, k].T
            res[f'k{k}'] = Wk
        E0a = np.zeros((2 * Hin, 2 * O)); E0a[Hin:, :O] = Wc[:, :, 0].T
        E0b = np.zeros((2 * Hin, 2 * O)); E0b[:Hin, O:] = Wc[:, :, 0].T
        E2a = np.zeros((2 * Hin, 2 * O)); E2a[Hin:, :O] = Wc[:, :, 2].T
        E2b = np.zeros((2 * Hin, 2 * O)); E2b[:Hin, O:] = Wc[:, :, 2].T
        res.update(e0a=E0a, e0b=E0b, e2a=E2a, e2b=E2b)
        return {k_: v.astype(np.float16) for k_, v in res.items()}
    convall = np.zeros((128, 28, 128), np.float16)
    for ni, (nm, key) in enumerate((('c1', 'conv1_w'), ('c9', 'conv9_w'),
                                    ('c16', 'conv16_w'), ('c17', 'conv17_w'))):
        cs = conv_stat(np.asarray(inputs[key], np.float64))
        for si, sfx in enumerate(('k0', 'k1', 'k2', 'e0a', 'e0b', 'e2a', 'e2b')):
            arr = cs[sfx]
            convall[:arr.shape[0], ni * 7 + si, :arr.shape[1]] = arr
    out['convall'] = np.ascontiguousarray(convall.reshape(128, 28 * 128))

    bn = np.zeros((H, 6), np.float32)
    for k, nm in enumerate(('bn1', 'bn9', 'bn16')):
        bn[:, 2 * k] = np.asarray(inputs[nm + '_g'], np.float32)
        bn[:, 2 * k + 1] = np.asarray(inputs[nm + '_b'], np.float32)
    out['bnp'] = bn
    return out


def _host_prep_x(x_shard):
    # x (BL,1,L) -> [(x, ci=1)=2 part, (b, ip, c)] fp16
    xs = np.asarray(x_shard, np.float32).reshape(BL, IP, 2, C)
    return np.ascontiguousarray(xs.transpose(2, 0, 1, 3), np.float16)


# ---------------------------------------------------------------------------
# Device kernel
# ---------------------------------------------------------------------------

_CACHE = {}

CONV_SFX = ('k0', 'k1', 'k2', 'e0a', 'e0b', 'e2a', 'e2b')


def _conv_layer(nc, P_ps, src, dst_raw, stats_out, w, dma_out=None):
    """conv k=3 pad=1 from h-orient src into raw (pre-BN) dst + bn_stats."""
    nparts = 2 if dma_out is not None else dst_raw.shape[0]
    for b in range(BL):
        for iph in range(2):
            ip0 = 4 * iph
            ips = slice(ip0, ip0 + 4)
            acc = P_ps.tile([nparts, 4, C], F32, tag="bigps")
            nc.tensor.matmul(acc[:, :, :], w['k1'], src[:, b, ips, :],
                             start=True, stop=False)
            nc.tensor.matmul(acc[:, :, 1:C], w['k0'],
                             src[:, b, ips, 0:C - 1], start=False, stop=False)
            nc.tensor.matmul(acc[:, :, 0:C - 1], w['k2'],
                             src[:, b, ips, 1:C], start=False, stop=False)
            if ip0 == 0:
                nc.tensor.matmul(acc[:, 1:4, 0:1], w['e0a'],
                                 src[:, b, 0:3, C - 1:C],
                                 start=False, stop=False)
            else:
                nc.tensor.matmul(acc[:, 0:4, 0:1], w['e0a'],
                                 src[:, b, ip0 - 1:ip0 + 3, C - 1:C],
                                 start=False, stop=False)
            nc.tensor.matmul(acc[:, 0:4, 0:1], w['e0b'],
                             src[:, b, ips, C - 1:C], start=False, stop=False)
            nc.tensor.matmul(acc[:, 0:4, C - 1:C], w['e2a'],
                             src[:, b, ips, 0:1], start=False, stop=False)
            if ip0 == 0:
                nc.tensor.matmul(acc[:, 0:4, C - 1:C], w['e2b'],
                                 src[:, b, 1:5, 0:1], start=False, stop=True)
            else:
                nc.tensor.matmul(acc[:, 0:3, C - 1:C], w['e2b'],
                                 src[:, b, ip0 + 1:ip0 + 4, 0:1],
                                 start=False, stop=True)
            if stats_out is not None:
                nc.vector.bn_stats(out=stats_out[:, 2 * b + iph, :],
                                   in_=acc.rearrange("p a c -> p (a c)"))
            if dma_out is not None:
                ev = dma_out[1].tile([2, 4, C], F32, tag="finev")
                nc.scalar.activation(out=ev, in_=acc, func=AF.Copy)
                nc.sync.dma_start(out=dma_out[0][:, b, ips, :], in_=ev)
            else:
                nc.scalar.activation(out=dst_raw[:, b, ips, :], in_=acc,
                                     func=AF.Copy)


def _bn_finalize(nc, scr, P_dram, stats, bnp, k, gb128):
    """stats [128,16,6] --bn_aggr/AllReduce--> gb128 [128,2]=(gamma',beta')."""
    mv = scr[:, 0:2]; pay = scr[:, 2:4]; red = scr[:, 4:6]
    pair = scr[0:H, 6:10].rearrange("p (x y) -> p x y", x=2)
    gm = scr[0:H, 10:12]; var = scr[0:H, 12:13]; eps_t = scr[0:H, 13:14]
    std = scr[0:H, 14:15]; rs = scr[0:H, 15:16]; gp = scr[0:H, 16:18]
    nc.vector.bn_aggr(out=mv, in_=stats)
    nc.vector.tensor_tensor(out=pay[:, 1:2], in0=mv[:, 0:1], in1=mv[:, 0:1],
                            op=OP.mult)
    nc.vector.tensor_tensor(out=pay[:, 1:2], in0=pay[:, 1:2], in1=mv[:, 1:2],
                            op=OP.add)
    nc.vector.tensor_copy(out=pay[:, 0:1], in_=mv[:, 0:1])
    cin = P_dram.tile([H2, 2], F32, tag=f"bnin{k}")
    cout = P_dram.tile([H2, 2], F32, tag=f"bnout{k}", addr_space="Shared")
    nc.sync.dma_start(out=cin, in_=pay)
    nc.gpsimd.collective_compute("AllReduce", OP.add, ins=[cin[:]],
                                 outs=[cout[:]],
                                 replica_groups=[list(range(NCORES))])
    nc.sync.dma_start(out=red, in_=cout)
    nc.sync.dma_start(out=pair[:, 0, :], in_=red[0:H, :])
    nc.sync.dma_start(out=pair[:, 1, :], in_=red[H:H2, :])
    nc.vector.tensor_tensor(out=gm, in0=pair[:, 0, :], in1=pair[:, 1, :],
                            op=OP.add)
    nc.vector.tensor_scalar_mul(gm, gm, 1.0 / (2 * NCORES))
    nc.vector.tensor_tensor(out=var, in0=gm[:, 0:1], in1=gm[:, 0:1],
                            op=OP.mult)
    nc.vector.tensor_tensor(out=var, in0=gm[:, 1:2], in1=var, op=OP.subtract)
    nc.vector.memset(eps_t, EPS)
    nc.scalar.activation(out=std, in_=var, func=AF.Sqrt, bias=eps_t)
    nc.vector.reciprocal(out=rs, in_=std)
    nc.vector.tensor_tensor(out=gp[:, 0:1], in0=bnp[:, 2 * k:2 * k + 1],
                            in1=rs, op=OP.mult)
    nc.vector.tensor_tensor(out=gp[:, 1:2], in0=gp[:, 0:1], in1=gm[:, 0:1],
                            op=OP.mult)
    nc.vector.tensor_tensor(out=gp[:, 1:2], in0=bnp[:, 2 * k + 1:2 * k + 2],
                            in1=gp[:, 1:2], op=OP.subtract)
    nc.sync.dma_start(out=gb128[0:H, :], in_=gp)
    nc.sync.dma_start(out=gb128[H:H2, :], in_=gp)


def _build():
    if 'nc' in _CACHE:
        return _CACHE['nc']
    import contextlib
    nc = bacc.Bacc("TRN2", target_bir_lowering=False, debug=False,
                   num_devices=NCORES)

    dram = {}
    def din(name, shape, dtype=F16):
        dram[name] = nc.dram_tensor(name, shape, dtype,
                                    kind="ExternalInput").ap()

    din('xq', [2, BL, IP, C])
    din('tmat', [C, NB, H * C]); din('vmat', [C, NB, H * N2])
    din('zvmat', [C, NB, H * N2])
    din('mmat', [N2, NB, H * C]); din('m2mat', [N2, NB, H * C])
    din('zrscan', [N2, NB, H]); din('ziscan', [N2, NB, H])
    din('wblk', [H2, NB, H2]); din('bout2', [H2, NB], F32)
    din('convall', [128, 28 * 128])
    din('bnp', [H, 6], F32)
    out_d = nc.dram_tensor('out', [2, BL, IP, C], F32,
                           kind="ExternalOutput").ap()

    with tile.TileContext(nc) as tc:
        ctx = contextlib.ExitStack()
        P_stat = ctx.enter_context(tc.tile_pool(name="stat", bufs=1))
        P_act = ctx.enter_context(tc.tile_pool(name="act", bufs=1))
        P_w = ctx.enter_context(tc.tile_pool(name="wstream", bufs=2))
        P_wz = ctx.enter_context(tc.tile_pool(name="wz", bufs=2))
        P_tmp = ctx.enter_context(tc.tile_pool(name="tmp", bufs=2))
        P_ps1 = ctx.enter_context(tc.tile_pool(name="ps1", bufs=2,
                                               space="PSUM"))
        P_ps2 = ctx.enter_context(tc.tile_pool(name="ps2", bufs=2,
                                               space="PSUM"))
        P_psy = ctx.enter_context(tc.tile_pool(name="psy", bufs=2,
                                               space="PSUM"))
        P_psw = ctx.enter_context(tc.tile_pool(name="psw", bufs=2,
                                               space="PSUM"))
        P_dram = ctx.enter_context(tc.tile_pool(name="cdram", bufs=1,
                                                space="DRAM"))

        # static tiles
        zr_s = P_stat.tile([N2, NB, H], F16)
        nc.sync.dma_start(out=zr_s, in_=dram['zrscan'])
        zi_s = P_stat.tile([N2, NB, H], F16)
        nc.sync.dma_start(out=zi_s, in_=dram['ziscan'])
        wblk_s = P_stat.tile([H2, NB, H2], F16)
        nc.sync.dma_start(out=wblk_s, in_=dram['wblk'])
        bout_s = P_stat.tile([H2, NB], F32)
        nc.sync.dma_start(out=bout_s, in_=dram['bout2'])
        bnp_s = P_stat.tile([H, 6], F32)
        nc.sync.dma_start(out=bnp_s, in_=dram['bnp'])
        xq_s = P_stat.tile([2, BL, IP, C], F16)
        nc.sync.dma_start(out=xq_s, in_=dram['xq'])
        convall_s = P_stat.tile([128, 28, 128], F16)
        nc.sync.dma_start(out=convall_s, in_=dram['convall'].rearrange(
            "p (k c) -> p k c", k=28))
        convw = {}
        for ni, nm in enumerate(('c1', 'c9', 'c16', 'c17')):
            npart = 2 if nm == 'c1' else H2
            ncol = 2 if nm == 'c17' else H2
            convw[nm] = {sfx: convall_s[0:npart, ni * 7 + si, 0:ncol]
                         for si, sfx in enumerate(CONV_SFX)}

        # activations
        uT = P_act.tile([H2, BL, IP, C], F16, tag="uT")
        uTn = P_act.tile([H2, BL, IP, C], F16, tag="uTn")
        u16c = P_act.tile([C, BL, IP, H2], F16, tag="u16c")
        g16c = P_act.tile([C, BL, IP, H2], F16, tag="g16c")
        gT = P_act.tile([H2, BL, IP, C], F16, tag="gT")
        # state slots: [sle0, c0, sle1, c1, ..., sle6, c6, sle7, pad]
        st = P_act.tile([N2, 16, H, BL], F16, tag="st")
        stv = st.rearrange("p (m two) h b -> p m two h b", two=2)
        gb128 = P_stat.tile([H2, 2], F32, tag="gb128")

        def u16c_dma(src_h):
            # h-orient [H2,(b,ip,c)] -> c-orient u16c, chunked by b-pairs
            for bp in range(0, BL, 2):
                nc.sync.dma_start_transpose(
                    u16c[:, bp:bp + 2].rearrange("c b i p -> c (b i) p"),
                    src_h[:, bp:bp + 2].rearrange("p b i c -> p (b i c)"))

        def s4_block(j, u_h, u_next):
            u5 = u16c.rearrange("c b i (x h) -> c b i x h", x=2)

            # ---- phase 1: states.  sle[m]=V@u_even; pi[m]=(ZV)@u_even
            #      + V@u_odd accumulated in psum; evict into slots. ----
            for hck in range(H // HC):
                h0 = HC * hck
                vm = P_wz.tile([C, HC, N2], F16, tag="vm")
                nc.sync.dma_start(out=vm, in_=dram['vmat'][:, j].rearrange(
                    "p (h n) -> p h n", h=H)[:, h0:h0 + HC, :])
                zv = P_wz.tile([C, HC, N2], F16, tag="zv")
                nc.sync.dma_start(out=zv, in_=dram['zvmat'][:, j].rearrange(
                    "p (h n) -> p h n", h=H)[:, h0:h0 + HC, :])
                for g in range(2):
                    hr = h0 + 8 * g
                    acc_s = P_ps1.tile([N2, 8, IP, BL], F32, tag="sle")
                    acc_p = P_ps2.tile([N2, 8, 7, BL], F32, tag="pi")
                    for hh in range(8):
                        lh = 8 * g + hh
                        nc.tensor.matmul(
                            acc_s[:, hh].rearrange("p m b -> p b m"),
                            vm[:, lh, :],
                            u5[:, :, :, 0, hr + hh],
                            start=True, stop=True)
                        nc.tensor.matmul(
                            acc_p[:, hh].rearrange("p m b -> p b m"),
                            vm[:, lh, :],
                            u5[:, :, 0:7, 1, hr + hh],
                            start=True, stop=False)
                        nc.tensor.matmul(
                            acc_p[:, hh].rearrange("p m b -> p b m"),
                            zv[:, lh, :],
                            u5[:, :, 0:7, 0, hr + hh],
                            start=False, stop=True)
                    nc.scalar.activation(
                        out=stv[:, :, 0, hr:hr + 8, :],
                        in_=acc_s.rearrange("p hh m b -> p m hh b"),
                        func=AF.Copy)
                    nc.scalar.activation(
                        out=stv[:, 0:7, 1, hr:hr + 8, :],
                        in_=acc_p.rearrange("p hh m b -> p m hh b"),
                        func=AF.Copy)

            # ---- phase 2: carry scan c[m] = z2 (.) c[m-1] + pi[m] ----
            zr_b = zr_s[:, j, :]
            zr_ap = bass.AP(tensor=zr_b.tensor, offset=zr_b.offset,
                            ap=[zr_b.ap[0], zr_b.ap[1], [0, BL]])
            zi_b = zi_s[0:N, j, :]          # +zi at base 0
            zi_lo = bass.AP(tensor=zi_b.tensor, offset=zi_b.offset,
                            ap=[zi_b.ap[0], zi_b.ap[1], [0, BL]])
            zi_c = zi_s[N:N2, j, :]         # -zi at base 64
            zi_hi = bass.AP(tensor=zi_c.tensor, offset=zi_c.offset,
                            ap=[zi_c.ap[0], zi_c.ap[1], [0, BL]])
            for m in range(1, 7):
                prev = stv[:, m - 1, 1]     # c[m-1]  [N2, H, BL]
                cur = stv[:, m, 1]          # pi[m] -> c[m]
                tsw = P_tmp.tile([N2, H, BL], F16, tag="tsw")
                tzr = P_tmp.tile([N2, H, BL], F16, tag="tzr")
                nc.vector.tensor_tensor(out=tsw[0:N], in0=prev[N:],
                                        in1=zi_hi, op=OP.mult)
                nc.vector.tensor_tensor(out=tsw[N:], in0=prev[0:N],
                                        in1=zi_lo, op=OP.mult)
                nc.vector.tensor_tensor(out=tzr, in0=prev,
                                        in1=zr_ap, op=OP.mult)
                nc.vector.tensor_tensor(out=tzr, in0=tzr, in1=tsw, op=OP.add)
                nc.vector.tensor_tensor(out=cur, in0=tzr, in1=cur, op=OP.add)

            # ---- phase 3: y = T@u + cross; gelu evict ----
            for hck in range(H // HC):
                h0 = HC * hck
                tm = P_w.tile([C, HC, C], F16, tag="tm")
                nc.sync.dma_start(out=tm, in_=dram['tmat'][:, j].rearrange(
                    "p (h c) -> p h c", h=H)[:, h0:h0 + HC, :])
                mm = P_w.tile([N2, HC, C], F16, tag="mm")
                nc.sync.dma_start(out=mm, in_=dram['mmat'][:, j].rearrange(
                    "p (h c) -> p h c", h=H)[:, h0:h0 + HC, :])
                m2 = P_w.tile([N2, HC, C], F16, tag="m2")
                nc.sync.dma_start(out=m2, in_=dram['m2mat'][:, j].rearrange(
                    "p (h c) -> p h c", h=H)[:, h0:h0 + HC, :])
                for g in range(4):
                    hr = h0 + 4 * g
                    # psum [C, b, l, hh]: l = 2*ip + x
                    acc = P_psy.tile([C, BL, NCH, 4], F32, tag="yps")
                    accl = acc.rearrange("c b (i x) hh -> c b i x hh", x=2)
                    for hh in range(4):
                        lh = 4 * g + hh
                        nc.tensor.matmul(accl[:, :, :, :, hh],
                                         tm[:, lh, :],
                                         u5[:, :, :, :, hr + hh],
                                         start=True, stop=False)
                        # slots 0..14 -> l = slot+1 (1..15)
                        nc.tensor.matmul(
                            acc[:, :, 1:16, hh].rearrange("c b l -> c l b"),
                            mm[:, lh, :],
                            st[:, 0:15, hr + hh, :],
                            start=False, stop=False)
                        # c[m'] m'=0..6 -> l = 2m'+3 (3,5,..,15)
                        nc.tensor.matmul(
                            accl[:, :, 1:8, 1, hh].rearrange("c b i -> c i b"),
                            m2[:, lh, :],
                            stv[:, 0:7, 1, hr + hh, :],
                            start=False, stop=True)
                    g5 = g16c.rearrange("c b i (x h) -> c b i x h", x=2)
                    nc.scalar.activation(
                        out=g5[:, :, :, :, hr:hr + 4],
                        in_=accl,
                        func=AF.Gelu)

            # ---- gT via xbar (b-pair chunks); Wout + bout + residual ----
            for bp in range(0, BL, 2):
                nc.sync.dma_start_transpose(
                    gT[:, bp:bp + 2].rearrange("p b i c -> p (b i) c"),
                    g16c[:, bp:bp + 2].rearrange("c b i p -> c (b i p)"))
            gT_f = gT.rearrange("p b i c -> p (b i c)")
            uh_f = u_h.rearrange("p b i c -> p (b i c)")
            un_f = u_next.rearrange("p b i c -> p (b i c)")
            for t in range(16):
                sl = slice(512 * t, 512 * (t + 1))
                acc = P_psw.tile([H2, 512], F32, tag="bigps")
                nc.tensor.matmul(acc, wblk_s[:, j, :], gT_f[:, sl],
                                 start=True, stop=True)
                nc.vector.scalar_tensor_tensor(
                    out=un_f[:, sl], in0=acc, scalar=bout_s[:, j:j + 1],
                    in1=uh_f[:, sl], op0=OP.add, op1=OP.add)
                if t % 4 == 3 and j != 6 and j != 12:
                    bp = (t // 4) * 2
                    nc.sync.dma_start_transpose(
                        u16c[:, bp:bp + 2].rearrange("c b i p -> c (b i) p"),
                        u_next[:, bp:bp + 2].rearrange(
                            "p b i c -> p (b i c)"))

        # ---------------- network ----------------
        bnscr = P_stat.tile([H2, 32], F32, tag="bnscr")
        stats = P_stat.tile([H2, 16, 6], F32, tag="stats")
        _conv_layer(nc, P_psw, xq_s, gT, stats, convw['c1'])
        _bn_finalize(nc, bnscr, P_dram, stats, bnp_s, 0, gb128)
        nc.scalar.activation(out=uT, in_=gT, func=AF.Relu,
                             bias=gb128[:, 1:2], scale=gb128[:, 0:1])
        u16c_dma(uT)
        cur, nxt = uT, uTn
        for j in range(7):
            s4_block(j, cur, nxt)
            cur, nxt = nxt, cur
        _conv_layer(nc, P_psw, cur, gT, stats, convw['c9'])
        _bn_finalize(nc, bnscr, P_dram, stats, bnp_s, 1, gb128)
        nc.scalar.activation(out=cur, in_=gT, func=AF.Relu,
                             bias=gb128[:, 1:2], scale=gb128[:, 0:1])
        u16c_dma(cur)
        for j in range(7, 13):
            s4_block(j, cur, nxt)
            cur, nxt = nxt, cur
        _conv_layer(nc, P_psw, cur, gT, stats, convw['c16'])
        _bn_finalize(nc, bnscr, P_dram, stats, bnp_s, 2, gb128)
        nc.scalar.activation(out=cur, in_=gT, func=AF.Relu,
                             bias=gb128[:, 1:2], scale=gb128[:, 0:1])
        _conv_layer(nc, P_psw, cur, None, None, convw['c17'],
                    dma_out=(out_d, P_tmp))
        ctx.close()

    nc.compile()
    _CACHE['nc'] = nc
    return nc


# ---------------------------------------------------------------------------
# Entry point
# ---------------------------------------------------------------------------

def kernel(**inputs):
    nc = _build()
    prep = _host_prep(inputs)
    x = np.asarray(inputs['x'], np.float32)
    in_maps = []
    for c in range(NCORES):
        m = dict(prep)
        m['xq'] = _host_prep_x(x[c * BL:(c + 1) * BL])
        in_maps.append(m)
    res = run_bass_kernel_spmd(nc, in_maps, core_ids=list(range(NCORES)))
    outs = []
    for c in range(NCORES):
        o = res.results[c]['out']              # [2, BL, IP, C]
        outs.append(o.transpose(1, 2, 0, 3).reshape(BL, 1, L))
    return np.ascontiguousarray(np.concatenate(outs, 0), np.float32)


# revision 5
# speedup vs baseline: 1.1459x; 1.0667x over previous
# Trainium2 Bass kernel for DnCNN+S4D (nn_DnCNN_S4_74182675137230).
#
# Data parallel over batch B=64 across 8 NeuronCores (BL=8 per core).
# The S4D FFT long-conv is computed exactly via a chunked state-space scan
# (chunk C=128, stride-2 carry):
#   per channel h:   sle[m]  = V @ u_{2m}                     (even states)
#                    pi[m]   = (Z V) @ u_{2m} ... wait, see below
#                    pi[m]   = (ZV) @ u_{2m} + V @ u_{2m+1}   (pair partial)
#                    c[m]    = z^{2C} (.) c[m-1] + pi[m]      (DVE scan)
#                    y_l     = T @ u_l                        (local Toeplitz)
#                    y_{2m+1}+= M @ sle[m] + M2 @ c[m-1]      (M2 = M z^C)
#                    y_{2m}  += M @ c[m-1]
# then gelu and the channel-mix Wout (+bout +residual); all matmuls fp16
# with fp32 PSUM accumulation.  T/V/ZV/M/M2 are weight-only host
# preprocessing.  Training-mode BN statistics are AllReduced across cores.
#
# State slots are interleaved [sle0, c0, sle1, c1, ..., sle7, pad] so the
# full cross contribution streams as a single 120-column matmul per h.
#
# Layouts (x = chunk parity i%2, ip = i//2, l = 2*ip+x, pos = l*128 + c):
#   h-orient: [(x,h)=128 part, (b=8, ip=8, c=128) free]   (convs, Wout, BN)
#   c-orient: [c=128 part, (b, ip, (x,h)=128) free]       (per-h S4 matmuls)
# Orientation swaps are xbar transpose DMAs chunked by b-pairs.

import numpy as np

import concourse.bass as bass
import concourse.bacc as bacc
import concourse.tile as tile
from concourse import mybir
from concourse.bass_utils import run_bass_kernel_spmd

F32 = mybir.dt.float32
F16 = mybir.dt.float16
AF = mybir.ActivationFunctionType
OP = mybir.AluOpType

NCORES = 8
B, H, N, L, NB = 64, 64, 64, 2048, 13
BL = B // NCORES          # 8 local batches
C = 128                   # chunk length
NCH = L // C              # 16 chunks
IP = NCH // 2             # 8 chunk pairs (= carry steps)
KAP = 256.0               # state scaling to keep fp16 range
EPS = 1e-5
H2 = 2 * H                # 128 = (x, h) partition extent
N2 = 2 * N                # 128 = (re/im, n) state extent
HC = 16                   # h-chunk for weight streaming


# ---------------------------------------------------------------------------
# Host-side weight preprocessing (numpy) -> fp16 device matrices
# ---------------------------------------------------------------------------

def _host_prep(inputs):
    out = {}
    log_dt = np.asarray(inputs['s4_log_dt'], np.float64)
    logA_re = np.asarray(inputs['s4_logA_re'], np.float64)
    A_im = np.asarray(inputs['s4_A_im'], np.float64)
    C_re = np.asarray(inputs['s4_C_re'], np.float64)
    C_im = np.asarray(inputs['s4_C_im'], np.float64)
    D = np.asarray(inputs['s4_D'], np.float64)
    Wout = np.asarray(inputs['s4_Wout'], np.float64)
    bout = np.asarray(inputs['s4_bout'], np.float64)

    dt = np.exp(log_dt)[:, :, None]
    A = -np.exp(logA_re) + 1j * A_im
    dtA = dt * A
    w = np.exp(dtA)                                            # (NB,H,N)
    Ct = (C_re + 1j * C_im) * (np.exp(dtA) - 1.0) / A

    cc = np.arange(C)
    P = w[..., None] ** np.arange(2 * C + 1)                   # (NB,H,N,2C+1)
    K = 2.0 * np.real(np.einsum('jhn,jhne->jhe', Ct, P[..., :C]))
    K[:, :, 0] += D                                            # D*u folded

    # T lhsT [c', (h, c)] with T[c,c'] = K[c-c']
    dmat = cc[None, :] - cc[:, None]                           # (c',c)
    Tl = np.where((dmat >= 0)[None, None],
                  np.take_along_axis(np.broadcast_to(K[:, :, None, :],
                                                     (NB, H, C, C)),
                                     np.clip(dmat, 0, C - 1)[None, None],
                                     axis=3), 0.0)             # (NB,H,c',c)
    out['tmat'] = np.ascontiguousarray(
        Tl.transpose(2, 0, 1, 3).reshape(C, NB, H * C), np.float16)

    # V lhsT [c', (h, 2n)]: V[(ri,n),c'] = [Re;Im](Ct w^(C-1-c'))/KAP
    VC = Ct[..., None] * P[..., (C - 1) - cc]                  # (NB,H,N,c')
    Vl = np.concatenate([VC.real, VC.imag], axis=2) / KAP      # (NB,H,2N,c')
    out['vmat'] = np.ascontiguousarray(
        Vl.transpose(3, 0, 1, 2).reshape(C, NB, H * N2), np.float16)

    # ZV lhsT: (z^C V) -> Ct w^(2C-1-c')/KAP
    ZC = Ct[..., None] * P[..., (2 * C - 1) - cc]              # (NB,H,N,c')
    Zl = np.concatenate([ZC.real, ZC.imag], axis=2) / KAP
    out['zvmat'] = np.ascontiguousarray(
        Zl.transpose(3, 0, 1, 2).reshape(C, NB, H * N2), np.float16)

    # M lhsT [(ri,n), (h, c)]: y_cross[c] = 2*KAP*[Re|-Im](w^{c+1}) . s
    MC = P[..., cc + 1]                                        # (NB,H,N,c)
    Ml = np.concatenate([2 * KAP * MC.real, -2 * KAP * MC.imag], axis=2)
    out['mmat'] = np.ascontiguousarray(
        Ml.transpose(0, 2, 1, 3).reshape(NB, N2, H * C).transpose(1, 0, 2),
        np.float16)

    # M2 = M z^C: w^{c+1+C}
    M2C = P[..., cc + 1 + C]                                   # (NB,H,N,c)
    M2l = np.concatenate([2 * KAP * M2C.real, -2 * KAP * M2C.imag], axis=2)
    out['m2mat'] = np.ascontiguousarray(
        M2l.transpose(0, 2, 1, 3).reshape(NB, N2, H * C).transpose(1, 0, 2),
        np.float16)

    # scan multipliers z2 = w^{2C}: tiles [(ri,n), NB, h]
    z2 = w ** (2 * C)
    zr = z2.real.transpose(0, 2, 1)                            # (NB,N,H)
    zi = z2.imag.transpose(0, 2, 1)
    out['zrscan'] = np.ascontiguousarray(
        np.concatenate([zr, zr], 1).transpose(1, 0, 2), np.float16)
    out['ziscan'] = np.ascontiguousarray(
        np.concatenate([zi, -zi], 1).transpose(1, 0, 2), np.float16)

    # Wout block-diag over x: lhsT[(x,h),(x',o)] = Wout[o,h] d_{x,x'}
    wblk = np.zeros((NB, H2, H2))
    WT = Wout.transpose(0, 2, 1)
    wblk[:, :H, :H] = WT; wblk[:, H:, H:] = WT
    out['wblk'] = np.ascontiguousarray(wblk.transpose(1, 0, 2), np.float16)
    out['bout2'] = np.ascontiguousarray(
        np.concatenate([bout, bout], 1).T, np.float32)         # [128, NB]

    # conv stationaries: per tap k block-diag over x, plus x-crossing
    # edge stationaries (out c=0 needs l-1 from the other parity, etc).
    def conv_stat(Wc):                                         # (O, Hin, 3)
        O = Wc.shape[0]; Hin = Wc.shape[1]
        res = {}
        for k in range(3):
            Wk = np.zeros((2 * Hin, 2 * O))
            Wk[:Hin, :O] = Wc[:, :, k].T; Wk[Hin:, O:] = Wc[:, :, k].T
            res[f'k{k}'] = Wk
        E0a = np.zeros((2 * Hin, 2 * O)); E0a[Hin:, :O] = Wc[:, :, 0].T
        E0b = np.zeros((2 * Hin, 2 * O)); E0b[:Hin, O:] = Wc[:, :, 0].T
        E2a = np.zeros((2 * Hin, 2 * O)); E2a[Hin:, :O] = Wc[:, :, 2].T
        E2b = np.zeros((2 * Hin, 2 * O)); E2b[:Hin, O:] = Wc[:, :, 2].T
        res.update(e0a=E0a, e0b=E0b, e2a=E2a, e2b=E2b)
        return {k_: v.astype(np.float16) for k_, v in res.items()}
    convall = np.zeros((128, 28, 128), np.float16)
    for ni, (nm, key) in enumerate((('c1', 'conv1_w'), ('c9', 'conv9_w'),
                                    ('c16', 'conv16_w'), ('c17', 'conv17_w'))):
        cs = conv_stat(np.asarray(inputs[key], np.float64))
        for si, sfx in enumerate(('k0', 'k1', 'k2', 'e0a', 'e0b', 'e2a', 'e2b')):
            arr = cs[sfx]
            convall[:arr.shape[0], ni * 7 + si, :arr.shape[1]] = arr
    out['convall'] = np.ascontiguousarray(convall.reshape(128, 28 * 128))

    bn = np.zeros((H, 6), np.float32)
    for k, nm in enumerate(('bn1', 'bn9', 'bn16')):
        bn[:, 2 * k] = np.asarray(inputs[nm + '_g'], np.float32)
        bn[:, 2 * k + 1] = np.asarray(inputs[nm + '_b'], np.float32)
    out['bnp'] = bn
    return out


def _host_prep_x(x_shard):
    # x (BL,1,L) -> [(x, ci=1)=2 part, (b, ip, c)] fp16
    xs = np.asarray(x_shard, np.float32).reshape(BL, IP, 2, C)
    return np.ascontiguousarray(xs.transpose(2, 0, 1, 3), np.float16)


# ---------------------------------------------------------------------------
# Device kernel
# ---------------------------------------------------------------------------

_CACHE = {}

CONV_SFX = ('k0', 'k1', 'k2', 'e0a', 'e0b', 'e2a', 'e2b')


def _conv_layer(nc, P_ps, src, dst_raw, stats_out, w, dma_out=None):
    """conv k=3 pad=1 from h-orient src into raw (pre-BN) dst + bn_stats."""
    nparts = 2 if dma_out is not None else dst_raw.shape[0]
    for b in range(BL):
        for iph in range(2):
            ip0 = 4 * iph
            ips = slice(ip0, ip0 + 4)
            acc = P_ps.tile([nparts, 4, C], F32, tag="bigps")
            nc.tensor.matmul(acc[:, :, :], w['k1'], src[:, b, ips, :],
                             start=True, stop=False)
            nc.tensor.matmul(acc[:, :, 1:C], w['k0'],
                             src[:, b, ips, 0:C - 1], start=False, stop=False)
            nc.tensor.matmul(acc[:, :, 0:C - 1], w['k2'],
                             src[:, b, ips, 1:C], start=False, stop=False)
            if ip0 == 0:
                nc.tensor.matmul(acc[:, 1:4, 0:1], w['e0a'],
                                 src[:, b, 0:3, C - 1:C],
                                 start=False, stop=False)
            else:
                nc.tensor.matmul(acc[:, 0:4, 0:1], w['e0a'],
                                 src[:, b, ip0 - 1:ip0 + 3, C - 1:C],
                                 start=False, stop=False)
            nc.tensor.matmul(acc[:, 0:4, 0:1], w['e0b'],
                             src[:, b, ips, C - 1:C], start=False, stop=False)
            nc.tensor.matmul(acc[:, 0:4, C - 1:C], w['e2a'],
                             src[:, b, ips, 0:1], start=False, stop=False)
            if ip0 == 0:
                nc.tensor.matmul(acc[:, 0:4, C - 1:C], w['e2b'],
                                 src[:, b, 1:5, 0:1], start=False, stop=True)
            else:
                nc.tensor.matmul(acc[:, 0:3, C - 1:C], w['e2b'],
                                 src[:, b, ip0 + 1:ip0 + 4, 0:1],
                                 start=False, stop=True)
            if stats_out is not None:
                nc.vector.bn_stats(out=stats_out[:, 2 * b + iph, :],
                                   in_=acc.rearrange("p a c -> p (a c)"))
            if dma_out is not None:
                ev = dma_out[1].tile([2, 4, C], F32, tag="finev")
                nc.scalar.activation(out=ev, in_=acc, func=AF.Copy)
                nc.sync.dma_start(out=dma_out[0][:, b, ips, :], in_=ev)
            else:
                nc.scalar.activation(out=dst_raw[:, b, ips, :], in_=acc,
                                     func=AF.Copy)


def _bn_finalize(nc, scr, P_dram, stats, bnp, k, gb128):
    """stats [128,16,6] --bn_aggr/AllReduce--> gb128 [128,2]=(gamma',beta')."""
    mv = scr[:, 0:2]; pay = scr[:, 2:4]; red = scr[:, 4:6]
    pair = scr[0:H, 6:10].rearrange("p (x y) -> p x y", x=2)
    gm = scr[0:H, 10:12]; var = scr[0:H, 12:13]; eps_t = scr[0:H, 13:14]
    std = scr[0:H, 14:15]; rs = scr[0:H, 15:16]; gp = scr[0:H, 16:18]
    nc.vector.bn_aggr(out=mv, in_=stats)
    nc.vector.tensor_tensor(out=pay[:, 1:2], in0=mv[:, 0:1], in1=mv[:, 0:1],
                            op=OP.mult)
    nc.vector.tensor_tensor(out=pay[:, 1:2], in0=pay[:, 1:2], in1=mv[:, 1:2],
                            op=OP.add)
    nc.vector.tensor_copy(out=pay[:, 0:1], in_=mv[:, 0:1])
    cin = P_dram.tile([H2, 2], F32, tag=f"bnin{k}")
    cout = P_dram.tile([H2, 2], F32, tag=f"bnout{k}", addr_space="Shared")
    nc.sync.dma_start(out=cin, in_=pay)
    nc.gpsimd.collective_compute("AllReduce", OP.add, ins=[cin[:]],
                                 outs=[cout[:]],
                                 replica_groups=[list(range(NCORES))])
    nc.sync.dma_start(out=red, in_=cout)
    nc.sync.dma_start(out=pair[:, 0, :], in_=red[0:H, :])
    nc.sync.dma_start(out=pair[:, 1, :], in_=red[H:H2, :])
    nc.vector.tensor_tensor(out=gm, in0=pair[:, 0, :], in1=pair[:, 1, :],
                            op=OP.add)
    nc.vector.tensor_scalar_mul(gm, gm, 1.0 / (2 * NCORES))
    nc.vector.tensor_tensor(out=var, in0=gm[:, 0:1], in1=gm[:, 0:1],
                            op=OP.mult)
    nc.vector.tensor_tensor(out=var, in0=gm[:, 1:2], in1=var, op=OP.subtract)
    nc.vector.memset(eps_t, EPS)
    nc.scalar.activation(out=std, in_=var, func=AF.Sqrt, bias=eps_t)
    nc.vector.reciprocal(out=rs, in_=std)
    nc.vector.tensor_tensor(out=gp[:, 0:1], in0=bnp[:, 2 * k:2 * k + 1],
                            in1=rs, op=OP.mult)
    nc.vector.tensor_tensor(out=gp[:, 1:2], in0=gp[:, 0:1], in1=gm[:, 0:1],
                            op=OP.mult)
    nc.vector.tensor_tensor(out=gp[:, 1:2], in0=bnp[:, 2 * k + 1:2 * k + 2],
                            in1=gp[:, 1:2], op=OP.subtract)
    nc.sync.dma_start(out=gb128[0:H, :], in_=gp)
    nc.sync.dma_start(out=gb128[H:H2, :], in_=gp)


def _build():
    if 'nc' in _CACHE:
        return _CACHE['nc']
    import contextlib
    nc = bacc.Bacc("TRN2", target_bir_lowering=False, debug=False,
                   num_devices=NCORES)

    dram = {}
    def din(name, shape, dtype=F16):
        dram[name] = nc.dram_tensor(name, shape, dtype,
                                    kind="ExternalInput").ap()

    din('xq', [2, BL, IP, C])
    din('tmat', [C, NB, H * C]); din('vmat', [C, NB, H * N2])
    din('zvmat', [C, NB, H * N2])
    din('mmat', [N2, NB, H * C]); din('m2mat', [N2, NB, H * C])
    din('zrscan', [N2, NB, H]); din('ziscan', [N2, NB, H])
    din('wblk', [H2, NB, H2]); din('bout2', [H2, NB], F32)
    din('convall', [128, 28 * 128])
    din('bnp', [H, 6], F32)
    out_d = nc.dram_tensor('out', [2, BL, IP, C], F32,
                           kind="ExternalOutput").ap()

    with tile.TileContext(nc) as tc:
        ctx = contextlib.ExitStack()
        P_stat = ctx.enter_context(tc.tile_pool(name="stat", bufs=1))
        P_act = ctx.enter_context(tc.tile_pool(name="act", bufs=1))
        P_w = ctx.enter_context(tc.tile_pool(name="wstream", bufs=2))
        P_wz = ctx.enter_context(tc.tile_pool(name="wz", bufs=2))
        P_tmp = ctx.enter_context(tc.tile_pool(name="tmp", bufs=2))
        P_ps1 = ctx.enter_context(tc.tile_pool(name="ps1", bufs=2,
                                               space="PSUM"))
        P_ps2 = ctx.enter_context(tc.tile_pool(name="ps2", bufs=2,
                                               space="PSUM"))
        P_psy = ctx.enter_context(tc.tile_pool(name="psy", bufs=2,
                                               space="PSUM"))
        P_psw = ctx.enter_context(tc.tile_pool(name="psw", bufs=2,
                                               space="PSUM"))
        P_dram = ctx.enter_context(tc.tile_pool(name="cdram", bufs=1,
                                                space="DRAM"))

        # static tiles
        zr_s = P_stat.tile([N2, NB, H], F16)
        nc.sync.dma_start(out=zr_s, in_=dram['zrscan'])
        zi_s = P_stat.tile([N2, NB, H], F16)
        nc.sync.dma_start(out=zi_s, in_=dram['ziscan'])
        wblk_s = P_stat.tile([H2, NB, H2], F16)
        nc.sync.dma_start(out=wblk_s, in_=dram['wblk'])
        bout_s = P_stat.tile([H2, NB], F32)
        nc.sync.dma_start(out=bout_s, in_=dram['bout2'])
        bnp_s = P_stat.tile([H, 6], F32)
        nc.sync.dma_start(out=bnp_s, in_=dram['bnp'])
        xq_s = P_stat.tile([2, BL, IP, C], F16)
        nc.sync.dma_start(out=xq_s, in_=dram['xq'])
        convall_s = P_stat.tile([128, 28, 128], F16)
        nc.sync.dma_start(out=convall_s, in_=dram['convall'].rearrange(
            "p (k c) -> p k c", k=28))
        convw = {}
        for ni, nm in enumerate(('c1', 'c9', 'c16', 'c17')):
            npart = 2 if nm == 'c1' else H2
            ncol = 2 if nm == 'c17' else H2
            convw[nm] = {sfx: convall_s[0:npart, ni * 7 + si, 0:ncol]
                         for si, sfx in enumerate(CONV_SFX)}

        # activations
        uT = P_act.tile([H2, BL, IP, C], F16, tag="uT")
        uTn = P_act.tile([H2, BL, IP, C], F16, tag="uTn")
        u16c = P_act.tile([C, BL, IP, H2], F16, tag="u16c")
        g16c = P_act.tile([C, BL, IP, H2], F16, tag="g16c")
        gT = P_act.tile([H2, BL, IP, C], F16, tag="gT")
        # state slots per h-half: [sle0, c0, sle1, c1, ..., sle7, pad]
        HH = H // 2
        st_lo = P_act.tile([N2, 16, HH, BL], F16, tag="stlo")
        st_hi = P_act.tile([N2, 16, HH, BL], F16, tag="sthi")
        sthalf = (st_lo, st_hi)
        gb128 = P_stat.tile([H2, 2], F32, tag="gb128")

        def u16c_dma(src_h):
            # h-orient [H2,(b,ip,c)] -> c-orient u16c, chunked by b-pairs
            for bp in range(0, BL, 2):
                nc.sync.dma_start_transpose(
                    u16c[:, bp:bp + 2].rearrange("c b i p -> c (b i) p"),
                    src_h[:, bp:bp + 2].rearrange("p b i c -> p (b i c)"))

        scan_engines = (nc.vector, nc.gpsimd)

        def s4_block(j, u_h, u_next):
            u5 = u16c.rearrange("c b i (x h) -> c b i x h", x=2)

            # ---- phase 1 + scan, by h-half ----
            for half in range(2):
                st = sthalf[half]
                stv = st.rearrange("p (m two) h b -> p m two h b", two=2)
                for hq in range(2):            # 16-h weight chunks
                    h0 = 32 * half + HC * hq
                    vm = P_wz.tile([C, HC, N2], F16, tag="vm")
                    nc.sync.dma_start(out=vm, in_=dram['vmat'][:, j].rearrange(
                        "p (h n) -> p h n", h=H)[:, h0:h0 + HC, :])
                    zv = P_wz.tile([C, HC, N2], F16, tag="zv")
                    nc.sync.dma_start(out=zv, in_=dram['zvmat'][:, j].rearrange(
                        "p (h n) -> p h n", h=H)[:, h0:h0 + HC, :])
                    for g in range(4):         # 4-h psum groups
                        hr = h0 + 4 * g
                        hl = hr - 32 * half    # h index within half tile
                        acc = P_ps1.tile([N2, 4, BL, IP, 2], F32, tag="sp")
                        for hh in range(4):
                            lh = 4 * g + hh
                            nc.tensor.matmul(acc[:, hh],
                                             vm[:, lh, :],
                                             u5[:, :, :, :, hr + hh],
                                             start=True, stop=False)
                            nc.tensor.matmul(acc[:, hh, :, 0:7, 1],
                                             zv[:, lh, :],
                                             u5[:, :, 0:7, 0, hr + hh],
                                             start=False, stop=True)
                        nc.scalar.activation(
                            out=stv[:, :, :, hl:hl + 4, :],
                            in_=acc.rearrange("p hh b m x -> p m x hh b"),
                            func=AF.Copy)
                # ---- carry scan on this half ----
                eng = scan_engines[half]
                hs = slice(32 * half, 32 * half + 32)
                zr_b = zr_s[:, j, hs]
                zr_ap = bass.AP(tensor=zr_b.tensor, offset=zr_b.offset,
                                ap=[zr_b.ap[0], zr_b.ap[1], [0, BL]])
                zi_b = zi_s[0:N, j, hs]
                zi_lo = bass.AP(tensor=zi_b.tensor, offset=zi_b.offset,
                                ap=[zi_b.ap[0], zi_b.ap[1], [0, BL]])
                zi_c = zi_s[N:N2, j, hs]
                zi_hi = bass.AP(tensor=zi_c.tensor, offset=zi_c.offset,
                                ap=[zi_c.ap[0], zi_c.ap[1], [0, BL]])
                for m in range(1, 7):
                    prev = stv[:, m - 1, 1]     # c[m-1]  [N2, HH, BL]
                    cur = stv[:, m, 1]          # pi[m] -> c[m]
                    tsw = P_tmp.tile([N2, HH, BL], F16, tag=f"tsw{half}")
                    tzr = P_tmp.tile([N2, HH, BL], F16, tag=f"tzr{half}")
                    eng.tensor_tensor(out=tsw[0:N], in0=prev[N:],
                                      in1=zi_hi, op=OP.mult)
                    eng.tensor_tensor(out=tsw[N:], in0=prev[0:N],
                                      in1=zi_lo, op=OP.mult)
                    eng.tensor_tensor(out=tzr, in0=prev,
                                      in1=zr_ap, op=OP.mult)
                    eng.tensor_tensor(out=tzr, in0=tzr, in1=tsw, op=OP.add)
                    eng.tensor_tensor(out=cur, in0=tzr, in1=cur, op=OP.add)

            # ---- phase 3: y = T@u + cross; gelu evict ----
            for hck in range(H // HC):
                h0 = HC * hck
                st = sthalf[h0 // 32]
                stv = st.rearrange("p (m two) h b -> p m two h b", two=2)
                tm = P_w.tile([C, HC, C], F16, tag="tm")
                nc.sync.dma_start(out=tm, in_=dram['tmat'][:, j].rearrange(
                    "p (h c) -> p h c", h=H)[:, h0:h0 + HC, :])
                mm = P_w.tile([N2, HC, C], F16, tag="mm")
                nc.sync.dma_start(out=mm, in_=dram['mmat'][:, j].rearrange(
                    "p (h c) -> p h c", h=H)[:, h0:h0 + HC, :])
                m2 = P_w.tile([N2, HC, C], F16, tag="m2")
                nc.sync.dma_start(out=m2, in_=dram['m2mat'][:, j].rearrange(
                    "p (h c) -> p h c", h=H)[:, h0:h0 + HC, :])
                for g in range(4):
                    hr = h0 + 4 * g
                    hl = hr % 32               # within half tile
                    # psum [C, b, l, hh]: l = 2*ip + x
                    acc = P_psy.tile([C, BL, NCH, 4], F32, tag="yps")
                    accl = acc.rearrange("c b (i x) hh -> c b i x hh", x=2)
                    for hh in range(4):
                        lh = 4 * g + hh
                        nc.tensor.matmul(accl[:, :, :, :, hh],
                                         tm[:, lh, :],
                                         u5[:, :, :, :, hr + hh],
                                         start=True, stop=False)
                        # slots 0..14 -> l = slot+1 (1..15)
                        nc.tensor.matmul(
                            acc[:, :, 1:16, hh].rearrange("c b l -> c l b"),
                            mm[:, lh, :],
                            st[:, 0:15, hl + hh, :],
                            start=False, stop=False)
                        # c[m'] m'=0..6 -> l = 2m'+3 (3,5,..,15)
                        nc.tensor.matmul(
                            accl[:, :, 1:8, 1, hh].rearrange("c b i -> c i b"),
                            m2[:, lh, :],
                            stv[:, 0:7, 1, hl + hh, :],
                            start=False, stop=True)
                    g5 = g16c.rearrange("c b i (x h) -> c b i x h", x=2)
                    nc.scalar.activation(
                        out=g5[:, :, :, :, hr:hr + 4],
                        in_=accl,
                        func=AF.Gelu)

            # ---- gT via xbar (b-pair chunks); Wout + bout + residual ----
            for bp in range(0, BL, 2):
                nc.sync.dma_start_transpose(
                    gT[:, bp:bp + 2].rearrange("p b i c -> p (b i) c"),
                    g16c[:, bp:bp + 2].rearrange("c b i p -> c (b i p)"))
            gT_f = gT.rearrange("p b i c -> p (b i c)")
            uh_f = u_h.rearrange("p b i c -> p (b i c)")
            un_f = u_next.rearrange("p b i c -> p (b i c)")
            for t in range(16):
                sl = slice(512 * t, 512 * (t + 1))
                acc = P_psw.tile([H2, 512], F32, tag="bigps")
                nc.tensor.matmul(acc, wblk_s[:, j, :], gT_f[:, sl],
                                 start=True, stop=True)
                nc.vector.scalar_tensor_tensor(
                    out=un_f[:, sl], in0=acc, scalar=bout_s[:, j:j + 1],
                    in1=uh_f[:, sl], op0=OP.add, op1=OP.add)
                if t % 4 == 3 and j != 6 and j != 12:
                    bp = (t // 4) * 2
                    nc.sync.dma_start_transpose(
                        u16c[:, bp:bp + 2].rearrange("c b i p -> c (b i) p"),
                        u_next[:, bp:bp + 2].rearrange(
                            "p b i c -> p (b i c)"))

        # ---------------- network ----------------
        bnscr = P_stat.tile([H2, 32], F32, tag="bnscr")
        stats = P_stat.tile([H2, 16, 6], F32, tag="stats")
        _conv_layer(nc, P_psw, xq_s, gT, stats, convw['c1'])
        _bn_finalize(nc, bnscr, P_dram, stats, bnp_s, 0, gb128)
        nc.scalar.activation(out=uT, in_=gT, func=AF.Relu,
                             bias=gb128[:, 1:2], scale=gb128[:, 0:1])
        u16c_dma(uT)
        cur, nxt = uT, uTn
        for j in range(7):
            s4_block(j, cur, nxt)
            cur, nxt = nxt, cur
        _conv_layer(nc, P_psw, cur, gT, stats, convw['c9'])
        _bn_finalize(nc, bnscr, P_dram, stats, bnp_s, 1, gb128)
        nc.scalar.activation(out=cur, in_=gT, func=AF.Relu,
                             bias=gb128[:, 1:2], scale=gb128[:, 0:1])
        u16c_dma(cur)
        for j in range(7, 13):
            s4_block(j, cur, nxt)
            cur, nxt = nxt, cur
        _conv_layer(nc, P_psw, cur, gT, stats, convw['c16'])
        _bn_finalize(nc, bnscr, P_dram, stats, bnp_s, 2, gb128)
        nc.scalar.activation(out=cur, in_=gT, func=AF.Relu,
                             bias=gb128[:, 1:2], scale=gb128[:, 0:1])
        _conv_layer(nc, P_psw, cur, None, None, convw['c17'],
                    dma_out=(out_d, P_tmp))
        ctx.close()

    nc.compile()
    _CACHE['nc'] = nc
    return nc


# ---------------------------------------------------------------------------
# Entry point
# ---------------------------------------------------------------------------

def kernel(**inputs):
    nc = _build()
    prep = _host_prep(inputs)
    x = np.asarray(inputs['x'], np.float32)
    in_maps = []
    for c in range(NCORES):
        m = dict(prep)
        m['xq'] = _host_prep_x(x[c * BL:(c + 1) * BL])
        in_maps.append(m)
    res = run_bass_kernel_spmd(nc, in_maps, core_ids=list(range(NCORES)))
    outs = []
    for c in range(NCORES):
        o = res.results[c]['out']              # [2, BL, IP, C]
        outs.append(o.transpose(1, 2, 0, 3).reshape(BL, 1, L))
    return np.ascontiguousarray(np.concatenate(outs, 0), np.float32)


# revision 7
# speedup vs baseline: 1.1478x; 1.0016x over previous
# Trainium2 Bass kernel for DnCNN+S4D (nn_DnCNN_S4_74182675137230).
#
# Data parallel over batch B=64 across 8 NeuronCores (BL=8 per core).
# The S4D FFT long-conv is computed exactly via a chunked state-space scan
# (chunk C=128, stride-2 carry):
#   per channel h:   sle[m]  = V @ u_{2m}                     (even states)
#                    pi[m]   = (Z V) @ u_{2m} ... wait, see below
#                    pi[m]   = (ZV) @ u_{2m} + V @ u_{2m+1}   (pair partial)
#                    c[m]    = z^{2C} (.) c[m-1] + pi[m]      (DVE scan)
#                    y_l     = T @ u_l                        (local Toeplitz)
#                    y_{2m+1}+= M @ sle[m] + M2 @ c[m-1]      (M2 = M z^C)
#                    y_{2m}  += M @ c[m-1]
# then gelu and the channel-mix Wout (+bout +residual); all matmuls fp16
# with fp32 PSUM accumulation.  T/V/ZV/M/M2 are weight-only host
# preprocessing.  Training-mode BN statistics are AllReduced across cores.
#
# State slots are interleaved [sle0, c0, sle1, c1, ..., sle7, pad] so the
# full cross contribution streams as a single 120-column matmul per h.
#
# Layouts (x = chunk parity i%2, ip = i//2, l = 2*ip+x, pos = l*128 + c):
#   h-orient: [(x,h)=128 part, (b=8, ip=8, c=128) free]   (convs, Wout, BN)
#   c-orient: [c=128 part, (b, ip, (x,h)=128) free]       (per-h S4 matmuls)
# Orientation swaps are xbar transpose DMAs chunked by b-pairs.

import numpy as np

import concourse.bass as bass
import concourse.bacc as bacc
import concourse.tile as tile
from concourse import mybir
from concourse.bass_utils import run_bass_kernel_spmd

F32 = mybir.dt.float32
F16 = mybir.dt.float16
AF = mybir.ActivationFunctionType
OP = mybir.AluOpType

NCORES = 8
B, H, N, L, NB = 64, 64, 64, 2048, 13
BL = B // NCORES          # 8 local batches
C = 128                   # chunk length
NCH = L // C              # 16 chunks
IP = NCH // 2             # 8 chunk pairs (= carry steps)
KAP = 256.0               # state scaling to keep fp16 range
EPS = 1e-5
H2 = 2 * H                # 128 = (x, h) partition extent
N2 = 2 * N                # 128 = (re/im, n) state extent
HC = 16                   # h-chunk for weight streaming


# ---------------------------------------------------------------------------
# Host-side weight preprocessing (numpy) -> fp16 device matrices
# ---------------------------------------------------------------------------

def _host_prep(inputs):
    out = {}
    log_dt = np.asarray(inputs['s4_log_dt'], np.float64)
    logA_re = np.asarray(inputs['s4_logA_re'], np.float64)
    A_im = np.asarray(inputs['s4_A_im'], np.float64)
    C_re = np.asarray(inputs['s4_C_re'], np.float64)
    C_im = np.asarray(inputs['s4_C_im'], np.float64)
    D = np.asarray(inputs['s4_D'], np.float64)
    Wout = np.asarray(inputs['s4_Wout'], np.float64)
    bout = np.asarray(inputs['s4_bout'], np.float64)

    dt = np.exp(log_dt)[:, :, None]
    A = -np.exp(logA_re) + 1j * A_im
    dtA = dt * A
    w = np.exp(dtA)                                            # (NB,H,N)
    Ct = (C_re + 1j * C_im) * (np.exp(dtA) - 1.0) / A

    cc = np.arange(C)
    P = w[..., None] ** np.arange(2 * C + 1)                   # (NB,H,N,2C+1)
    K = 2.0 * np.real(np.einsum('jhn,jhne->jhe', Ct, P[..., :C]))
    K[:, :, 0] += D                                            # D*u folded

    # T lhsT [c', (h, c)] with T[c,c'] = K[c-c']
    dmat = cc[None, :] - cc[:, None]                           # (c',c)
    Tl = np.where((dmat >= 0)[None, None],
                  np.take_along_axis(np.broadcast_to(K[:, :, None, :],
                                                     (NB, H, C, C)),
                                     np.clip(dmat, 0, C - 1)[None, None],
                                     axis=3), 0.0)             # (NB,H,c',c)
    out['tmat'] = np.ascontiguousarray(
        Tl.transpose(2, 0, 1, 3).reshape(C, NB, H * C), np.float16)

    # V lhsT [c', (h, 2n)]: V[(ri,n),c'] = [Re;Im](Ct w^(C-1-c'))/KAP
    VC = Ct[..., None] * P[..., (C - 1) - cc]                  # (NB,H,N,c')
    Vl = np.concatenate([VC.real, VC.imag], axis=2) / KAP      # (NB,H,2N,c')
    out['vmat'] = np.ascontiguousarray(
        Vl.transpose(3, 0, 1, 2).reshape(C, NB, H * N2), np.float16)

    # ZV lhsT: (z^C V) -> Ct w^(2C-1-c')/KAP
    ZC = Ct[..., None] * P[..., (2 * C - 1) - cc]              # (NB,H,N,c')
    Zl = np.concatenate([ZC.real, ZC.imag], axis=2) / KAP
    out['zvmat'] = np.ascontiguousarray(
        Zl.transpose(3, 0, 1, 2).reshape(C, NB, H * N2), np.float16)

    # M lhsT [(ri,n), (h, c)]: y_cross[c] = 2*KAP*[Re|-Im](w^{c+1}) . s
    MC = P[..., cc + 1]                                        # (NB,H,N,c)
    Ml = np.concatenate([2 * KAP * MC.real, -2 * KAP * MC.imag], axis=2)
    out['mmat'] = np.ascontiguousarray(
        Ml.transpose(0, 2, 1, 3).reshape(NB, N2, H * C).transpose(1, 0, 2),
        np.float16)

    # M2 = M z^C: w^{c+1+C}
    M2C = P[..., cc + 1 + C]                                   # (NB,H,N,c)
    M2l = np.concatenate([2 * KAP * M2C.real, -2 * KAP * M2C.imag], axis=2)
    out['m2mat'] = np.ascontiguousarray(
        M2l.transpose(0, 2, 1, 3).reshape(NB, N2, H * C).transpose(1, 0, 2),
        np.float16)

    # scan multipliers z2 = w^{2C}: tiles [(ri,n), NB, h]
    z2 = w ** (2 * C)
    zr = z2.real.transpose(0, 2, 1)                            # (NB,N,H)
    zi = z2.imag.transpose(0, 2, 1)
    out['zrscan'] = np.ascontiguousarray(
        np.concatenate([zr, zr], 1).transpose(1, 0, 2), np.float16)
    out['ziscan'] = np.ascontiguousarray(
        np.concatenate([zi, -zi], 1).transpose(1, 0, 2), np.float16)

    # Wout block-diag over x: lhsT[(x,h),(x',o)] = Wout[o,h] d_{x,x'}
    wblk = np.zeros((NB, H2, H2))
    WT = Wout.transpose(0, 2, 1)
    wblk[:, :H, :H] = WT; wblk[:, H:, H:] = WT
    out['wblk'] = np.ascontiguousarray(wblk.transpose(1, 0, 2), np.float16)
    out['bout2'] = np.ascontiguousarray(
        np.concatenate([bout, bout], 1).T, np.float32)         # [128, NB]

    # conv stationaries: per tap k block-diag over x, plus x-crossing
    # edge stationaries (out c=0 needs l-1 from the other parity, etc).
    def conv_stat(Wc):                                         # (O, Hin, 3)
        O = Wc.shape[0]; Hin = Wc.shape[1]
        res = {}
        for k in range(3):
            Wk = np.zeros((2 * Hin, 2 * O))
            Wk[:Hin, :O] = Wc[:, :, k].T; Wk[Hin:, O:] = Wc[:, :, k].T
            res[f'k{k}'] = Wk
        E0a = np.zeros((2 * Hin, 2 * O)); E0a[Hin:, :O] = Wc[:, :, 0].T
        E0b = np.zeros((2 * Hin, 2 * O)); E0b[:Hin, O:] = Wc[:, :, 0].T
        E2a = np.zeros((2 * Hin, 2 * O)); E2a[Hin:, :O] = Wc[:, :, 2].T
        E2b = np.zeros((2 * Hin, 2 * O)); E2b[:Hin, O:] = Wc[:, :, 2].T
        res.update(e0a=E0a, e0b=E0b, e2a=E2a, e2b=E2b)
        return {k_: v.astype(np.float16) for k_, v in res.items()}
    convall = np.zeros((128, 28, 128), np.float16)
    for ni, (nm, key) in enumerate((('c1', 'conv1_w'), ('c9', 'conv9_w'),
                                    ('c16', 'conv16_w'), ('c17', 'conv17_w'))):
        cs = conv_stat(np.asarray(inputs[key], np.float64))
        for si, sfx in enumerate(('k0', 'k1', 'k2', 'e0a', 'e0b', 'e2a', 'e2b')):
            arr = cs[sfx]
            convall[:arr.shape[0], ni * 7 + si, :arr.shape[1]] = arr
    out['convall'] = np.ascontiguousarray(convall.reshape(128, 28 * 128))

    bn = np.zeros((H, 6), np.float32)
    for k, nm in enumerate(('bn1', 'bn9', 'bn16')):
        bn[:, 2 * k] = np.asarray(inputs[nm + '_g'], np.float32)
        bn[:, 2 * k + 1] = np.asarray(inputs[nm + '_b'], np.float32)
    out['bnp'] = bn
    return out


def _host_prep_x(x_shard):
    # x (BL,1,L) -> [(x, ci=1)=2 part, (b, ip, c)] fp16
    xs = np.asarray(x_shard, np.float32).reshape(BL, IP, 2, C)
    return np.ascontiguousarray(xs.transpose(2, 0, 1, 3), np.float16)


# ---------------------------------------------------------------------------
# Device kernel
# ---------------------------------------------------------------------------

_CACHE = {}

CONV_SFX = ('k0', 'k1', 'k2', 'e0a', 'e0b', 'e2a', 'e2b')


def _conv_layer(nc, P_ps, src, dst_raw, stats_out, w, dma_out=None):
    """conv k=3 pad=1 from h-orient src into raw (pre-BN) dst + bn_stats."""
    nparts = 2 if dma_out is not None else dst_raw.shape[0]
    for b in range(BL):
        for iph in range(2):
            ip0 = 4 * iph
            ips = slice(ip0, ip0 + 4)
            acc = P_ps.tile([nparts, 4, C], F32, tag="bigps")
            nc.tensor.matmul(acc[:, :, :], w['k1'], src[:, b, ips, :],
                             start=True, stop=False)
            nc.tensor.matmul(acc[:, :, 1:C], w['k0'],
                             src[:, b, ips, 0:C - 1], start=False, stop=False)
            nc.tensor.matmul(acc[:, :, 0:C - 1], w['k2'],
                             src[:, b, ips, 1:C], start=False, stop=False)
            if ip0 == 0:
                nc.tensor.matmul(acc[:, 1:4, 0:1], w['e0a'],
                                 src[:, b, 0:3, C - 1:C],
                                 start=False, stop=False)
            else:
                nc.tensor.matmul(acc[:, 0:4, 0:1], w['e0a'],
                                 src[:, b, ip0 - 1:ip0 + 3, C - 1:C],
                                 start=False, stop=False)
            nc.tensor.matmul(acc[:, 0:4, 0:1], w['e0b'],
                             src[:, b, ips, C - 1:C], start=False, stop=False)
            nc.tensor.matmul(acc[:, 0:4, C - 1:C], w['e2a'],
                             src[:, b, ips, 0:1], start=False, stop=False)
            if ip0 == 0:
                nc.tensor.matmul(acc[:, 0:4, C - 1:C], w['e2b'],
                                 src[:, b, 1:5, 0:1], start=False, stop=True)
            else:
                nc.tensor.matmul(acc[:, 0:3, C - 1:C], w['e2b'],
                                 src[:, b, ip0 + 1:ip0 + 4, 0:1],
                                 start=False, stop=True)
            if stats_out is not None:
                nc.vector.bn_stats(out=stats_out[:, 2 * b + iph, :],
                                   in_=acc.rearrange("p a c -> p (a c)"))
            if dma_out is not None:
                ev = dma_out[1].tile([2, 4, C], F32, tag="finev")
                nc.scalar.activation(out=ev, in_=acc, func=AF.Copy)
                nc.sync.dma_start(out=dma_out[0][:, b, ips, :], in_=ev)
            else:
                nc.scalar.activation(out=dst_raw[:, b, ips, :], in_=acc,
                                     func=AF.Copy)


def _bn_finalize(nc, scr, P_dram, stats, bnp, k, gb128):
    """stats [128,16,6] --bn_aggr/AllReduce--> gb128 [128,2]=(gamma',beta')."""
    mv = scr[:, 0:2]; pay = scr[:, 2:4]; red = scr[:, 4:6]
    pair = scr[0:H, 6:10].rearrange("p (x y) -> p x y", x=2)
    gm = scr[0:H, 10:12]; var = scr[0:H, 12:13]; eps_t = scr[0:H, 13:14]
    std = scr[0:H, 14:15]; rs = scr[0:H, 15:16]; gp = scr[0:H, 16:18]
    nc.vector.bn_aggr(out=mv, in_=stats)
    nc.vector.tensor_tensor(out=pay[:, 1:2], in0=mv[:, 0:1], in1=mv[:, 0:1],
                            op=OP.mult)
    nc.vector.tensor_tensor(out=pay[:, 1:2], in0=pay[:, 1:2], in1=mv[:, 1:2],
                            op=OP.add)
    nc.vector.tensor_copy(out=pay[:, 0:1], in_=mv[:, 0:1])
    cin = P_dram.tile([H2, 2], F32, tag=f"bnin{k}")
    cout = P_dram.tile([H2, 2], F32, tag=f"bnout{k}", addr_space="Shared")
    nc.sync.dma_start(out=cin, in_=pay)
    nc.gpsimd.collective_compute("AllReduce", OP.add, ins=[cin[:]],
                                 outs=[cout[:]],
                                 replica_groups=[list(range(NCORES))])
    nc.sync.dma_start(out=red, in_=cout)
    nc.sync.dma_start(out=pair[:, 0, :], in_=red[0:H, :])
    nc.sync.dma_start(out=pair[:, 1, :], in_=red[H:H2, :])
    nc.vector.tensor_tensor(out=gm, in0=pair[:, 0, :], in1=pair[:, 1, :],
                            op=OP.add)
    nc.vector.tensor_scalar_mul(gm, gm, 1.0 / (2 * NCORES))
    nc.vector.tensor_tensor(out=var, in0=gm[:, 0:1], in1=gm[:, 0:1],
                            op=OP.mult)
    nc.vector.tensor_tensor(out=var, in0=gm[:, 1:2], in1=var, op=OP.subtract)
    nc.vector.memset(eps_t, EPS)
    nc.scalar.activation(out=std, in_=var, func=AF.Sqrt, bias=eps_t)
    nc.vector.reciprocal(out=rs, in_=std)
    nc.vector.tensor_tensor(out=gp[:, 0:1], in0=bnp[:, 2 * k:2 * k + 1],
                            in1=rs, op=OP.mult)
    nc.vector.tensor_tensor(out=gp[:, 1:2], in0=gp[:, 0:1], in1=gm[:, 0:1],
                            op=OP.mult)
    nc.vector.tensor_tensor(out=gp[:, 1:2], in0=bnp[:, 2 * k + 1:2 * k + 2],
                            in1=gp[:, 1:2], op=OP.subtract)
    nc.sync.dma_start(out=gb128[0:H, :], in_=gp)
    nc.sync.dma_start(out=gb128[H:H2, :], in_=gp)


def _build():
    if 'nc' in _CACHE:
        return _CACHE['nc']
    import contextlib
    nc = bacc.Bacc("TRN2", target_bir_lowering=False, debug=False,
                   num_devices=NCORES)

    dram = {}
    def din(name, shape, dtype=F16):
        dram[name] = nc.dram_tensor(name, shape, dtype,
                                    kind="ExternalInput").ap()

    din('xq', [2, BL, IP, C])
    din('tmat', [C, NB, H * C]); din('vmat', [C, NB, H * N2])
    din('zvmat', [C, NB, H * N2])
    din('mmat', [N2, NB, H * C]); din('m2mat', [N2, NB, H * C])
    din('zrscan', [N2, NB, H]); din('ziscan', [N2, NB, H])
    din('wblk', [H2, NB, H2]); din('bout2', [H2, NB], F32)
    din('convall', [128, 28 * 128])
    din('bnp', [H, 6], F32)
    out_d = nc.dram_tensor('out', [2, BL, IP, C], F32,
                           kind="ExternalOutput").ap()

    with tile.TileContext(nc) as tc:
        ctx = contextlib.ExitStack()
        P_stat = ctx.enter_context(tc.tile_pool(name="stat", bufs=1))
        P_act = ctx.enter_context(tc.tile_pool(name="act", bufs=1))
        P_w = ctx.enter_context(tc.tile_pool(name="wstream", bufs=2))
        P_wz = ctx.enter_context(tc.tile_pool(name="wz", bufs=2))
        P_tmp = ctx.enter_context(tc.tile_pool(name="tmp", bufs=2))
        P_ps1 = ctx.enter_context(tc.tile_pool(name="ps1", bufs=2,
                                               space="PSUM"))
        P_ps2 = ctx.enter_context(tc.tile_pool(name="ps2", bufs=2,
                                               space="PSUM"))
        P_psy = ctx.enter_context(tc.tile_pool(name="psy", bufs=2,
                                               space="PSUM"))
        P_psw = ctx.enter_context(tc.tile_pool(name="psw", bufs=2,
                                               space="PSUM"))
        P_dram = ctx.enter_context(tc.tile_pool(name="cdram", bufs=1,
                                                space="DRAM"))

        # static tiles
        zr_s = P_stat.tile([N2, NB, H], F16)
        nc.sync.dma_start(out=zr_s, in_=dram['zrscan'])
        zi_s = P_stat.tile([N2, NB, H], F16)
        nc.sync.dma_start(out=zi_s, in_=dram['ziscan'])
        wblk_s = P_stat.tile([H2, NB, H2], F16)
        nc.sync.dma_start(out=wblk_s, in_=dram['wblk'])
        bout_s = P_stat.tile([H2, NB], F32)
        nc.sync.dma_start(out=bout_s, in_=dram['bout2'])
        bnp_s = P_stat.tile([H, 6], F32)
        nc.sync.dma_start(out=bnp_s, in_=dram['bnp'])
        xq_s = P_stat.tile([2, BL, IP, C], F16)
        nc.sync.dma_start(out=xq_s, in_=dram['xq'])
        convall_s = P_stat.tile([128, 28, 128], F16)
        nc.sync.dma_start(out=convall_s, in_=dram['convall'].rearrange(
            "p (k c) -> p k c", k=28))
        convw = {}
        for ni, nm in enumerate(('c1', 'c9', 'c16', 'c17')):
            npart = 2 if nm == 'c1' else H2
            ncol = 2 if nm == 'c17' else H2
            convw[nm] = {sfx: convall_s[0:npart, ni * 7 + si, 0:ncol]
                         for si, sfx in enumerate(CONV_SFX)}

        # activations
        uT = P_act.tile([H2, BL, IP, C], F16, tag="uT")
        uTn = P_act.tile([H2, BL, IP, C], F16, tag="uTn")
        u16c_a = P_act.tile([C, BL, IP, H2], F16, tag="u16ca")
        u16c_b = P_act.tile([C, BL, IP, H2], F16, tag="u16cb")
        g16c = P_act.tile([C, BL, IP, H2], F16, tag="g16c")
        gT = P_act.tile([H2, BL, IP, C], F16, tag="gT")
        # state slots per h-half: [sle0, c0, sle1, c1, ..., sle7, pad]
        HH = H // 2
        st_lo = P_act.tile([N2, 16, HH, BL], F16, tag="stlo")
        st_hi = P_act.tile([N2, 16, HH, BL], F16, tag="sthi")
        sthalf = (st_lo, st_hi)
        gb128 = P_stat.tile([H2, 2], F32, tag="gb128")

        def u16c_dma(dst, src_h):
            # h-orient [H2,(b,ip,c)] -> c-orient, chunked by b-pairs
            for bp in range(0, BL, 2):
                nc.sync.dma_start_transpose(
                    dst[:, bp:bp + 2].rearrange("c b i p -> c (b i) p"),
                    src_h[:, bp:bp + 2].rearrange("p b i c -> p (b i c)"))

        scan_engines = (nc.vector, nc.gpsimd)

        def s4_block(j, u_h, u_next):
            u16c = u16c_a if j % 2 == 0 else u16c_b
            u16c_nxt = u16c_b if j % 2 == 0 else u16c_a
            u5 = u16c.rearrange("c b i (x h) -> c b i x h", x=2)

            # ---- phase 1 + scan, by h-half ----
            for half in range(2):
                st = sthalf[half]
                stv = st.rearrange("p (m two) h b -> p m two h b", two=2)
                for hq in range(2):            # 16-h weight chunks
                    h0 = 32 * half + HC * hq
                    vm = P_wz.tile([C, HC, N2], F16, tag="vm")
                    nc.sync.dma_start(out=vm, in_=dram['vmat'][:, j].rearrange(
                        "p (h n) -> p h n", h=H)[:, h0:h0 + HC, :])
                    zv = P_wz.tile([C, HC, N2], F16, tag="zv")
                    nc.sync.dma_start(out=zv, in_=dram['zvmat'][:, j].rearrange(
                        "p (h n) -> p h n", h=H)[:, h0:h0 + HC, :])
                    for g in range(4):         # 4-h psum groups
                        hr = h0 + 4 * g
                        hl = hr - 32 * half    # h index within half tile
                        acc = P_ps1.tile([N2, 4, BL, IP, 2], F32, tag="sp")
                        for hh in range(4):
                            lh = 4 * g + hh
                            nc.tensor.matmul(acc[:, hh],
                                             vm[:, lh, :],
                                             u5[:, :, :, :, hr + hh],
                                             start=True, stop=False)
                            nc.tensor.matmul(acc[:, hh, :, 0:7, 1],
                                             zv[:, lh, :],
                                             u5[:, :, 0:7, 0, hr + hh],
                                             start=False, stop=True)
                        nc.scalar.activation(
                            out=stv[:, :, :, hl:hl + 4, :],
                            in_=acc.rearrange("p hh b m x -> p m x hh b"),
                            func=AF.Copy)
                # ---- carry scan on this half ----
                eng = scan_engines[half]
                hs = slice(32 * half, 32 * half + 32)
                zr_b = zr_s[:, j, hs]
                zr_ap = bass.AP(tensor=zr_b.tensor, offset=zr_b.offset,
                                ap=[zr_b.ap[0], zr_b.ap[1], [0, BL]])
                zi_b = zi_s[0:N, j, hs]
                zi_lo = bass.AP(tensor=zi_b.tensor, offset=zi_b.offset,
                                ap=[zi_b.ap[0], zi_b.ap[1], [0, BL]])
                zi_c = zi_s[N:N2, j, hs]
                zi_hi = bass.AP(tensor=zi_c.tensor, offset=zi_c.offset,
                                ap=[zi_c.ap[0], zi_c.ap[1], [0, BL]])
                for m in range(1, 7):
                    prev = stv[:, m - 1, 1]     # c[m-1]  [N2, HH, BL]
                    cur = stv[:, m, 1]          # pi[m] -> c[m]
                    tsw = P_tmp.tile([N2, HH, BL], F16, tag=f"tsw{half}")
                    tzr = P_tmp.tile([N2, HH, BL], F16, tag=f"tzr{half}")
                    eng.tensor_tensor(out=tsw[0:N], in0=prev[N:],
                                      in1=zi_hi, op=OP.mult)
                    eng.tensor_tensor(out=tsw[N:], in0=prev[0:N],
                                      in1=zi_lo, op=OP.mult)
                    eng.tensor_tensor(out=tzr, in0=prev,
                                      in1=zr_ap, op=OP.mult)
                    eng.tensor_tensor(out=tzr, in0=tzr, in1=tsw, op=OP.add)
                    eng.tensor_tensor(out=cur, in0=tzr, in1=cur, op=OP.add)

            # ---- phase 3: y = T@u + cross; gelu evict ----
            for hck in range(H // HC):
                h0 = HC * hck
                st = sthalf[h0 // 32]
                stv = st.rearrange("p (m two) h b -> p m two h b", two=2)
                tm = P_w.tile([C, HC, C], F16, tag="tm")
                nc.sync.dma_start(out=tm, in_=dram['tmat'][:, j].rearrange(
                    "p (h c) -> p h c", h=H)[:, h0:h0 + HC, :])
                mm = P_w.tile([N2, HC, C], F16, tag="mm")
                nc.sync.dma_start(out=mm, in_=dram['mmat'][:, j].rearrange(
                    "p (h c) -> p h c", h=H)[:, h0:h0 + HC, :])
                m2 = P_w.tile([N2, HC, C], F16, tag="m2")
                nc.sync.dma_start(out=m2, in_=dram['m2mat'][:, j].rearrange(
                    "p (h c) -> p h c", h=H)[:, h0:h0 + HC, :])
                for g in range(4):
                    hr = h0 + 4 * g
                    hl = hr % 32               # within half tile
                    # psum [C, b, l, hh]: l = 2*ip + x
                    acc = P_psy.tile([C, BL, NCH, 4], F32, tag="yps")
                    accl = acc.rearrange("c b (i x) hh -> c b i x hh", x=2)
                    for hh in range(4):
                        lh = 4 * g + hh
                        nc.tensor.matmul(accl[:, :, :, :, hh],
                                         tm[:, lh, :],
                                         u5[:, :, :, :, hr + hh],
                                         start=True, stop=False)
                        # slots 0..14 -> l = slot+1 (1..15)
                        nc.tensor.matmul(
                            acc[:, :, 1:16, hh].rearrange("c b l -> c l b"),
                            mm[:, lh, :],
                            st[:, 0:15, hl + hh, :],
                            start=False, stop=False)
                        # c[m'] m'=0..6 -> l = 2m'+3 (3,5,..,15)
                        nc.tensor.matmul(
                            accl[:, :, 1:8, 1, hh].rearrange("c b i -> c i b"),
                            m2[:, lh, :],
                            stv[:, 0:7, 1, hl + hh, :],
                            start=False, stop=True)
                    g5 = g16c.rearrange("c b i (x h) -> c b i x h", x=2)
                    nc.scalar.activation(
                        out=g5[:, :, :, :, hr:hr + 4],
                        in_=accl,
                        func=AF.Gelu)

            # ---- gT via xbar (b-pair chunks); Wout + bout + residual ----
            for bp in range(0, BL, 2):
                nc.sync.dma_start_transpose(
                    gT[:, bp:bp + 2].rearrange("p b i c -> p (b i) c"),
                    g16c[:, bp:bp + 2].rearrange("c b i p -> c (b i p)"))
            gT_f = gT.rearrange("p b i c -> p (b i c)")
            uh_f = u_h.rearrange("p b i c -> p (b i c)")
            un_f = u_next.rearrange("p b i c -> p (b i c)")
            for t in range(16):
                sl = slice(512 * t, 512 * (t + 1))
                acc = P_psw.tile([H2, 512], F32, tag="bigps")
                nc.tensor.matmul(acc, wblk_s[:, j, :], gT_f[:, sl],
                                 start=True, stop=True)
                nc.vector.scalar_tensor_tensor(
                    out=un_f[:, sl], in0=acc, scalar=bout_s[:, j:j + 1],
                    in1=uh_f[:, sl], op0=OP.add, op1=OP.add)
                if t % 4 == 3 and j != 6 and j != 12:
                    bp = (t // 4) * 2
                    nc.sync.dma_start_transpose(
                        u16c_nxt[:, bp:bp + 2].rearrange("c b i p -> c (b i) p"),
                        u_next[:, bp:bp + 2].rearrange(
                            "p b i c -> p (b i c)"))

        # ---------------- network ----------------
        bnscr = P_stat.tile([H2, 32], F32, tag="bnscr")
        stats = P_stat.tile([H2, 16, 6], F32, tag="stats")
        _conv_layer(nc, P_psw, xq_s, gT, stats, convw['c1'])
        _bn_finalize(nc, bnscr, P_dram, stats, bnp_s, 0, gb128)
        nc.scalar.activation(out=uT, in_=gT, func=AF.Relu,
                             bias=gb128[:, 1:2], scale=gb128[:, 0:1])
        u16c_dma(u16c_a, uT)
        cur, nxt = uT, uTn
        for j in range(7):
            s4_block(j, cur, nxt)
            cur, nxt = nxt, cur
        _conv_layer(nc, P_psw, cur, gT, stats, convw['c9'])
        _bn_finalize(nc, bnscr, P_dram, stats, bnp_s, 1, gb128)
        nc.scalar.activation(out=cur, in_=gT, func=AF.Relu,
                             bias=gb128[:, 1:2], scale=gb128[:, 0:1])
        u16c_dma(u16c_b, cur)
        for j in range(7, 13):
            s4_block(j, cur, nxt)
            cur, nxt = nxt, cur
        _conv_layer(nc, P_psw, cur, gT, stats, convw['c16'])
        _bn_finalize(nc, bnscr, P_dram, stats, bnp_s, 2, gb128)
        nc.scalar.activation(out=cur, in_=gT, func=AF.Relu,
                             bias=gb128[:, 1:2], scale=gb128[:, 0:1])
        _conv_layer(nc, P_psw, cur, None, None, convw['c17'],
                    dma_out=(out_d, P_tmp))
        ctx.close()

    nc.compile()
    _CACHE['nc'] = nc
    return nc


# ---------------------------------------------------------------------------
# Entry point
# ---------------------------------------------------------------------------

def kernel(**inputs):
    nc = _build()
    prep = _host_prep(inputs)
    x = np.asarray(inputs['x'], np.float32)
    in_maps = []
    for c in range(NCORES):
        m = dict(prep)
        m['xq'] = _host_prep_x(x[c * BL:(c + 1) * BL])
        in_maps.append(m)
    res = run_bass_kernel_spmd(nc, in_maps, core_ids=list(range(NCORES)))
    outs = []
    for c in range(NCORES):
        o = res.results[c]['out']              # [2, BL, IP, C]
        outs.append(o.transpose(1, 2, 0, 3).reshape(BL, 1, L))
    return np.ascontiguousarray(np.concatenate(outs, 0), np.float32)
